# revision 2
# baseline (speedup 1.0000x reference)
"""GCN message-passing kernel for 8 Trainium2 NeuronCores.

Strategy (graph/data parallel, per the sharding hint):
  - Destination nodes are sharded across the 8 cores in contiguous ranges.
  - Within each core, its destinations are dealt (by in-degree, snake order)
    into 128-wide blocks so per-block edge counts are balanced across
    blocks AND cores (the SPMD program has compile-time-fixed loop bounds).
  - Per layer: each core computes hs = dinv * (x W^T + b) for its own node
    shard (PE transpose + matmul), downcasts to bf16, and the shards are
    AllGathered into a full [C*NPAD, 128] bf16 table in DRAM.
  - Messages are fetched with batched indirect DMA gathers (one SWDGE
    instruction per ~hundred 128-edge tiles) and scatter-added per
    destination block with a one-hot matmul:
        agg_block[d, f] += S_tile[e, d]^T @ msg_tile[e, f]
    accumulated in PSUM. S_tile is built on the DVE with a single
    is_equal tensor_scalar of an iota row against per-edge dest ranks.
  - BN statistics (sum, sum of squares) are computed with mask-vector
    matmuls over the aggregated blocks and AllReduced across cores; the
    apply (scale/shift + relu + residual) runs on full-shard DVE/ACT ops.

kernel(**inputs) takes the FULL inputs and returns the FULL output.
"""

import numpy as np
import ml_dtypes

import concourse.bacc as bacc
import concourse.bass as bass
import concourse.mybir as mybir
import concourse.tile as tile
from concourse.bass_utils import run_bass_kernel_spmd
from concourse.masks import make_identity

P = 128
F32 = mybir.dt.float32
BF16 = mybir.dt.bfloat16
AF = mybir.ActivationFunctionType
ALU = mybir.AluOpType


class Cfg:
    def __init__(self, N, E, D, L, C, bpc, kg=4, bn_eps=1e-5):
        assert D == 128
        self.N, self.E, self.D, self.L, self.C = N, E, D, L, C
        self.NSH = N // C                      # real nodes per core
        assert self.NSH * C == N
        self.TPC = (self.NSH + P - 1) // P     # node tiles (blocks) per core
        self.NPAD = self.TPC * P               # padded nodes per core
        assert self.NSH < self.NPAD, "need at least one guaranteed-zero pad row"
        self.TROWS = C * self.NPAD             # gather table rows
        self.BPC = bpc                         # blocks per gather chunk
        self.chunks = [
            list(range(i, min(i + bpc, self.TPC))) for i in range(0, self.TPC, bpc)
        ]
        self.BN_EPS = bn_eps
        self.KG = kg  # max idxs per dma_gather call (in 128-edge tiles)
        self.ZROW = self.NSH  # core 0's first pad row: always written as zero
        self.LO = 32768
        if self.TROWS > self.LO:
            c_hi = -((self.LO - self.NSH) // -self.NPAD)
            zhi = c_hi * self.NPAD + self.NSH
            assert self.LO <= zhi < self.TROWS
            self.ZHI = zhi - self.LO
        else:
            self.ZHI = 0


def _preprocess(cfg, x, edge_index, W, b, gamma, beta):
    """All index/layout work on the host. Returns per-core in_maps and the
    (identical across cores) compile-time tile structure."""
    N, C, NSH, NPAD, TPC = cfg.N, cfg.C, cfg.NSH, cfg.NPAD, cfg.TPC
    row = np.asarray(edge_index[0], dtype=np.int64)
    col = np.asarray(edge_index[1], dtype=np.int64)
    x = np.asarray(x, dtype=np.float32)
    deg = np.bincount(row, minlength=N).astype(np.float32)  # out-degree
    deg_in = np.bincount(col, minlength=N)

    # Per-core local permutation: snake-deal destinations (sorted by
    # in-degree desc) into TPC blocks -> balanced per-block edge counts.
    newlocal = np.empty(N, np.int64)
    nblk0 = None
    for c in range(C):
        ids = np.arange(c * NSH, (c + 1) * NSH)
        order = ids[np.argsort(-deg_in[ids], kind="stable")]
        i = np.arange(NSH)
        r, j = i // TPC, i % TPC
        blk = np.where(r % 2 == 1, TPC - 1 - j, j)
        rank = np.zeros(NSH, np.int64)
        cnt = np.zeros(TPC, np.int64)
        for k in range(NSH):
            rank[k] = cnt[blk[k]]
            cnt[blk[k]] += 1
        newlocal[order] = blk * P + rank
        if nblk0 is None:
            nblk0 = cnt.copy()
        else:
            assert (cnt == nblk0).all()
    assert nblk0.max() <= P

    maskv = (np.arange(P)[:, None] < nblk0[None, :]).astype(np.float32)
    table_row = (np.arange(N) // NSH) * NPAD + newlocal  # node -> table row

    e_core = col // NSH
    e_blk = newlocal[col] // P
    e_rank = newlocal[col] % P
    e_src = table_row[row]

    # common tile structure: TLs/THs tiles per block, max over cores/blocks
    split_hi = cfg.TROWS > cfg.LO
    per = {}
    TLs, THs = 1, (1 if split_hi else 0)
    for c in range(C):
        selc = e_core == c
        for lo in (True, False):
            if not lo and not split_hi:
                continue
            sel = selc & ((e_src < cfg.LO) == lo)
            srcs, blks, ranks = e_src[sel], e_blk[sel], e_rank[sel]
            o = np.argsort(blks, kind="stable")
            srcs, blks, ranks = srcs[o], blks[o], ranks[o]
            starts = np.searchsorted(blks, np.arange(TPC))
            ends = np.searchsorted(blks, np.arange(TPC) + 1)
            per[(c, lo)] = (srcs, ranks, starts, ends)
            m = int((-((ends - starts) // -P)).max())
            if lo:
                TLs = max(TLs, m)
            else:
                THs = max(THs, m)
    if not split_hi:
        per = {(c, True): per[(c, True)] for c in range(C)}
    TS = TLs + THs
    NT = TPC * TS
    in_maps = []
    Wt = np.ascontiguousarray(np.transpose(np.asarray(W, np.float32), (0, 2, 1)))
    bT = np.ascontiguousarray(np.asarray(b, np.float32).T)

    def _wrap16(idx):
        w = idx.reshape(-1, 16).T.astype(np.int16)
        return np.ascontiguousarray(np.tile(w, (8, 1)))

    for c in range(C):
        idx_lo = np.full(TPC * TLs * P, cfg.ZROW, np.int64)
        idx_hi = np.full(max(TPC * THs * P, 16), cfg.ZHI, np.int64)
        # one-hot S matrices, block-contiguous: smat[e, (b*TS + t)*P + d]
        smat = np.zeros((P, NT * P), ml_dtypes.bfloat16)
        lo_off = hi_off = 0
        for ch in cfg.chunks:
            for bidx in ch:
                srcs, ranks, st, en = per[(c, True)]
                cnt = en[bidx] - st[bidx]
                idx_lo[lo_off : lo_off + cnt] = srcs[st[bidx]:en[bidx]]
                pos = np.arange(cnt)
                rr = ranks[st[bidx]:en[bidx]]
                smat[pos % P, (bidx * TS + pos // P) * P + rr] = 1.0
                lo_off += TLs * P
            for bidx in ch:
                if THs == 0:
                    continue
                srcs, ranks, st, en = per[(c, False)]
                cnt = en[bidx] - st[bidx]
                idx_hi[hi_off : hi_off + cnt] = srcs[st[bidx]:en[bidx]] - cfg.LO
                pos = np.arange(cnt)
                rr = ranks[st[bidx]:en[bidx]]
                smat[pos % P, (bidx * TS + TLs + pos // P) * P + rr] = 1.0
                hi_off += THs * P

        ids = np.arange(c * NSH, (c + 1) * NSH)
        xin = np.zeros((NPAD, cfg.D), np.float32)
        xin[newlocal[ids]] = x[ids]
        degT = np.zeros((P, TPC), np.float32)
        degT[newlocal[ids] % P, newlocal[ids] // P] = deg[ids]

        in_maps.append(
            {
                "xin": xin,
                "wt": Wt,
                "bT": bT,
                "gamma": np.asarray(gamma, np.float32),
                "beta": np.asarray(beta, np.float32),
                "degT": degT,
                "maskv": maskv,
                "smat": smat,
                "idx_lo": _wrap16(idx_lo),
                "idx_hi": _wrap16(idx_hi),
            }
        )

    meta = dict(TLs=TLs, THs=THs, NT=NT, newlocal=newlocal)
    return in_maps, meta


def _build(cfg, TLs, THs):
    """Build the SPMD Bass program (identical for all cores)."""
    N, D, L, C = cfg.N, cfg.D, cfg.L, cfg.C
    TPC, NPAD, TROWS = cfg.TPC, cfg.NPAD, cfg.TROWS
    TS = TLs + THs
    NT = TPC * TS
    NTC_MAX = max(len(ch) for ch in cfg.chunks) * TS

    nc = bacc.Bacc("TRN2", target_bir_lowering=False, debug=False, num_devices=C)

    xin = nc.dram_tensor("xin", [NPAD, D], F32, kind="ExternalInput")
    wt = nc.dram_tensor("wt", [L, D, D], F32, kind="ExternalInput")
    bT = nc.dram_tensor("bT", [D, L], F32, kind="ExternalInput")
    gamma_d = nc.dram_tensor("gamma", [L, D], F32, kind="ExternalInput")
    beta_d = nc.dram_tensor("beta", [L, D], F32, kind="ExternalInput")
    degT = nc.dram_tensor("degT", [P, TPC], F32, kind="ExternalInput")
    maskv_d = nc.dram_tensor("maskv", [P, TPC], F32, kind="ExternalInput")
    smat_d = nc.dram_tensor("smat", [P, NT * P], BF16, kind="ExternalInput")
    idx_lo_d = nc.dram_tensor(
        "idx_lo", [P, TPC * TLs * P // 16], mybir.dt.int16, kind="ExternalInput"
    )
    nhi16 = max(TPC * THs * P, 16) // 16
    idx_hi_d = nc.dram_tensor(
        "idx_hi", [P, nhi16], mybir.dt.int16, kind="ExternalInput"
    )
    out_d = nc.dram_tensor("out", [NPAD, D], F32, kind="ExternalOutput")

    rg = [list(range(C))]

    with tile.TileContext(nc) as tc:
        with (
            tc.tile_pool(name="persist", bufs=1) as pp,
            tc.tile_pool(name="msgp", bufs=max(2, 112 // cfg.KG)) as msgp,
            tc.tile_pool(name="bigtmp", bufs=1) as btp,
            tc.tile_pool(name="sp", bufs=4) as sp,
            tc.tile_pool(name="work", bufs=4) as wp,
            tc.tile_pool(name="psblk", bufs=2, space="PSUM") as psblk,
            tc.tile_pool(name="psmisc", bufs=3, space="PSUM") as psmisc,
            tc.tile_pool(name="psbc", bufs=1, space="PSUM") as psbc,
            tc.tile_pool(name="psstat", bufs=2, space="PSUM") as psstat,
            tc.tile_pool(name="dram", bufs=1, space="DRAM") as dp,
        ):
            # ---- persistent loads ----
            x_sb = pp.tile([P, TPC, D], F32)
            nc.sync.dma_start(x_sb[:], xin[:].rearrange("(t p) f -> p t f", p=P))
            wt_sb = pp.tile([P, L, D], F32)
            for l in range(L):
                nc.sync.dma_start(wt_sb[:, l, :], wt[l, :, :])
            bT_sb = pp.tile([P, L], F32)
            nc.sync.dma_start(bT_sb[:], bT[:])
            gb_sb = pp.tile([1, 2 * L, D], F32)  # gamma/beta rows on partition 0
            for l in range(L):
                nc.sync.dma_start(gb_sb[:, l, :], gamma_d[l : l + 1, :])
                nc.sync.dma_start(gb_sb[:, L + l, :], beta_d[l : l + 1, :])
            deg_sb = pp.tile([P, TPC], F32)
            nc.sync.dma_start(deg_sb[:], degT[:])
            maskv_sb = pp.tile([P, TPC], F32)
            nc.sync.dma_start(maskv_sb[:], maskv_d[:])
            idx_lo_sb = pp.tile([P, TPC * TLs * P // 16], mybir.dt.int16)
            nc.sync.dma_start(idx_lo_sb[:], idx_lo_d[:])
            idx_hi_sb = pp.tile([P, nhi16], mybir.dt.int16)
            nc.sync.dma_start(idx_hi_sb[:], idx_hi_d[:])
            ident = pp.tile([P, P], F32)
            make_identity(nc, ident[:])
            ones1 = pp.tile([1, P], F32)
            nc.vector.memset(ones1[:], 1.0)

            # dinv = (deg > 0) / sqrt(max(deg, 1))
            dinv_sb = pp.tile([P, TPC], F32)
            t_a = wp.tile([P, TPC], F32, tag="dinv")
            nc.vector.tensor_scalar_max(t_a[:], deg_sb[:], 1.0)
            t_b = wp.tile([P, TPC], F32, tag="dinv")
            nc.vector.reciprocal(t_b[:], t_a[:])
            t_c = wp.tile([P, TPC], F32, tag="dinv")
            nc.scalar.sqrt(t_c[:], t_b[:])
            t_d = wp.tile([P, TPC], F32, tag="dinv")
            nc.vector.tensor_scalar(t_d[:], deg_sb[:], 0.0, None, ALU.is_gt)
            nc.vector.tensor_tensor(dinv_sb[:], t_c[:], t_d[:], ALU.mult)

            agg_sb = pp.tile([P, TPC, D], F32)
            hs_sb = pp.tile([P, TPC, D], BF16)

            # DRAM collective buffers
            shard_dr = dp.tile([NPAD, D], BF16)
            table_dr = dp.tile([TROWS, D], BF16)
            stats_in = dp.tile([1, 2 * D], F32)
            stats_out = dp.tile([1, 2 * D], F32)

            for l in range(L):
                # ---- hs = dinv * (x @ W^T + b), downcast bf16, row-major ----
                for t in range(TPC):
                    xT_ps = psmisc.tile([P, P], F32, tag="ps")
                    nc.tensor.transpose(xT_ps[:], x_sb[:, t, :], ident[:])
                    xT = wp.tile([P, P], F32, tag="xT")
                    nc.vector.tensor_copy(xT[:], xT_ps[:])
                    hT_ps = psmisc.tile([P, P], F32, tag="ps")
                    nc.tensor.matmul(
                        out=hT_ps[:], lhsT=wt_sb[:, l, :], rhs=xT[:],
                        start=True, stop=True,
                    )
                    hb = wp.tile([P, P], F32, tag="hb")
                    nc.scalar.activation(
                        hb[:], hT_ps[:], AF.Identity, bias=bT_sb[:, l : l + 1]
                    )
                    h_rm_ps = psmisc.tile([P, P], F32, tag="ps")
                    nc.tensor.transpose(h_rm_ps[:], hb[:], ident[:])
                    nc.scalar.activation(
                        hs_sb[:, t, :], h_rm_ps[:], AF.Identity,
                        scale=dinv_sb[:, t : t + 1],
                    )
                nc.sync.dma_start(
                    shard_dr[:].rearrange("(t p) f -> p t f", p=P), hs_sb[:]
                )
                nc.gpsimd.collective_compute(
                    "AllGather",
                    ALU.bypass,
                    ins=[shard_dr.opt()],
                    outs=[table_dr.opt()],
                    replica_groups=rg,
                )

                # ---- gather + one-hot matmul aggregation ----
                stA_ps = psstat.tile([1, P], F32, tag="st")
                stB_ps = psstat.tile([1, P], F32, tag="st")
                tile_col = 0
                lo_off = hi_off = 0
                for ch in cfg.chunks:
                    nb = len(ch)
                    ntc = nb * TS
                    # one msg tile per gather call (KG tiles each) for deep
                    # DMA pipelining via the pool; slot_of maps a chunk-local
                    # msg-tile column to its (pool tile, slot)
                    slot_of = {}

                    def _mt(mcol, _s=None):
                        mt, sl = slot_of[mcol]
                        return mt[:, sl, :]

                    nlo = nb * TLs * P
                    KGP = cfg.KG * P
                    for g0 in range(0, nlo, KGP):
                        g1 = min(g0 + KGP, nlo)
                        mt = msgp.tile([P, cfg.KG, D], BF16, tag="msg")
                        for i in range((g1 - g0) // P):
                            slot_of[g0 // P + i] = (mt, i)
                        nc.gpsimd.dma_gather(
                            mt[:, : (g1 - g0) // P, :],
                            table_dr[:],
                            idx_lo_sb[:, (lo_off + g0) // 16 : (lo_off + g1) // 16],
                            g1 - g0, g1 - g0, D,
                        )
                    lo_off += nlo
                    if THs > 0:
                        nhi = nb * THs * P
                        for g0 in range(0, nhi, KGP):
                            g1 = min(g0 + KGP, nhi)
                            mt = msgp.tile([P, cfg.KG, D], BF16, tag="msg")
                            for i in range((g1 - g0) // P):
                                slot_of[nb * TLs + g0 // P + i] = (mt, i)
                            nc.gpsimd.dma_gather(
                                mt[:, : (g1 - g0) // P, :],
                                table_dr[cfg.LO :, :],
                                idx_hi_sb[
                                    :, (hi_off + g0) // 16 : (hi_off + g1) // 16
                                ],
                                g1 - g0, g1 - g0, D,
                            )
                        hi_off += nhi
                    for j, bidx in enumerate(ch):
                        ps_b = psblk.tile([P, P], F32, tag="blk")
                        s_blk = sp.tile([P, TS, P], BF16, tag="s")
                        nc.sync.dma_start(
                            s_blk[:],
                            smat_d[:, bidx * TS * P : (bidx + 1) * TS * P],
                        )
                        mm, nmm = 0, TS
                        for t in range(TLs):
                            mcol = j * TLs + t
                            nc.tensor.matmul(
                                out=ps_b[:], lhsT=s_blk[:, t, :], rhs=_mt(mcol),
                                start=(mm == 0), stop=(mm == nmm - 1),
                            )
                            mm += 1
                        for t in range(THs):
                            mcol = nb * TLs + j * THs + t
                            nc.tensor.matmul(
                                out=ps_b[:], lhsT=s_blk[:, TLs + t, :], rhs=_mt(mcol),
                                start=(mm == 0), stop=(mm == nmm - 1),
                            )
                            mm += 1
                        nc.scalar.activation(
                            agg_sb[:, bidx, :], ps_b[:], AF.Identity,
                            scale=dinv_sb[:, bidx : bidx + 1],
                        )
                        nc.tensor.matmul(
                            out=stA_ps[:],
                            lhsT=maskv_sb[:, bidx : bidx + 1],
                            rhs=agg_sb[:, bidx, :],
                            start=(bidx == 0), stop=(bidx == TPC - 1),
                            skip_group_check=True,
                        )
                        aggsq = wp.tile([P, P], F32, tag="aggsq")
                        nc.scalar.square(aggsq[:], agg_sb[:, bidx, :])
                        nc.tensor.matmul(
                            out=stB_ps[:],
                            lhsT=maskv_sb[:, bidx : bidx + 1],
                            rhs=aggsq[:],
                            start=(bidx == 0), stop=(bidx == TPC - 1),
                            skip_group_check=True,
                        )
                    tile_col += ntc

                st_sb = wp.tile([1, 2, P], F32, tag="st")
                nc.vector.tensor_copy(st_sb[:, 0, :], stA_ps[:])
                nc.vector.tensor_copy(st_sb[:, 1, :], stB_ps[:])
                nc.sync.dma_start(stats_in[:], st_sb[:])
                nc.gpsimd.collective_compute(
                    "AllReduce",
                    ALU.add,
                    ins=[stats_in.opt()],
                    outs=[stats_out.opt()],
                    replica_groups=rg,
                )
                stg = wp.tile([1, 2, P], F32, tag="st")
                nc.sync.dma_start(stg[:], stats_out[:])

                # ---- scale/shift vectors on partition 0 ----
                vec = wp.tile([1, 8, P], F32, tag="vec")
                MU, MSQ, VAR, RSTD, SC, SH, T0, T1 = range(8)
                inv_n = 1.0 / float(N)
                nc.vector.tensor_scalar_mul(vec[:, MU, :], stg[:, 0, :], inv_n)
                nc.vector.tensor_scalar_mul(vec[:, MSQ, :], stg[:, 1, :], inv_n)
                nc.vector.tensor_tensor(
                    vec[:, T0, :], vec[:, MU, :], vec[:, MU, :], ALU.mult
                )
                nc.vector.tensor_tensor(
                    vec[:, VAR, :], vec[:, MSQ, :], vec[:, T0, :], ALU.subtract
                )
                nc.vector.tensor_scalar_add(vec[:, T1, :], vec[:, VAR, :], cfg.BN_EPS)
                nc.vector.reciprocal(vec[:, T0, :], vec[:, T1, :])
                nc.scalar.sqrt(vec[:, RSTD, :], vec[:, T0, :])
                nc.vector.tensor_tensor(
                    vec[:, SC, :], gb_sb[:, l, :], vec[:, RSTD, :], ALU.mult
                )
                nc.vector.tensor_tensor(
                    vec[:, T0, :], vec[:, MU, :], vec[:, SC, :], ALU.mult
                )
                nc.vector.tensor_tensor(
                    vec[:, SH, :], gb_sb[:, L + l, :], vec[:, T0, :], ALU.subtract
                )
                # broadcast scale|shift across partitions via ones-matmul
                bc_ps = psbc.tile([P, 2 * P], F32, tag="bc")
                nc.tensor.matmul(
                    out=bc_ps[:], lhsT=ones1[:], rhs=vec[:, SC : SH + 1, :],
                    start=True, stop=True,
                )
                screp = wp.tile([P, 2, P], F32, tag="screp")
                nc.vector.tensor_copy(screp[:], bc_ps[:])

                # ---- BN apply + relu + residual (whole shard) ----
                t1 = btp.tile([P, TPC, D], F32, tag="t1")
                nc.vector.tensor_tensor(
                    t1[:],
                    agg_sb[:],
                    screp[:, 0:1, :].to_broadcast([P, TPC, D]),
                    ALU.mult,
                )
                nc.vector.tensor_tensor(
                    t1[:],
                    t1[:],
                    screp[:, 1:2, :].to_broadcast([P, TPC, D]),
                    ALU.add,
                )
                nc.scalar.activation(t1[:], t1[:], AF.Relu)
                nc.vector.tensor_tensor(x_sb[:], x_sb[:], t1[:], ALU.add)

            nc.sync.dma_start(out_d[:].rearrange("(t p) f -> p t f", p=P), x_sb[:])

    nc.compile()
    return nc


_CACHE = {}


def _get_nc(cfg, TLs, THs):
    key = (cfg.N, cfg.E, cfg.L, cfg.C, cfg.BPC, cfg.KG, TLs, THs)
    if key not in _CACHE:
        _CACHE[key] = _build(cfg, TLs, THs)
    return _CACHE[key]


def run(cfg, inputs, trace=False):
    in_maps, meta = _preprocess(cfg, **inputs)
    nc = _get_nc(cfg, meta["TLs"], meta["THs"])
    res = run_bass_kernel_spmd(nc, in_maps, core_ids=list(range(cfg.C)), trace=trace)
    newlocal = meta["newlocal"]
    xfull = np.empty((cfg.N, cfg.D), np.float32)
    for c in range(cfg.C):
        ids = np.arange(c * cfg.NSH, (c + 1) * cfg.NSH)
        xfull[ids] = res.results[c]["out"][newlocal[ids]]
    return xfull, res


def kernel(x, edge_index, W, b, gamma, beta):
    cfg = Cfg(N=50000, E=800000, D=128, L=3, C=8, bpc=7, kg=8)
    out, _ = run(
        cfg, dict(x=x, edge_index=edge_index, W=W, b=b, gamma=gamma, beta=beta)
    )
    return out



# revision 4
# speedup vs baseline: 1.0833x; 1.0833x over previous
"""GCN message-passing kernel for 8 Trainium2 NeuronCores.

Strategy (graph/data parallel, per the sharding hint):
  - Destination nodes are sharded across the 8 cores in contiguous ranges.
  - Within each core, its destinations are dealt (by in-degree, snake order)
    into 128-wide blocks so per-block edge counts are balanced across
    blocks AND cores (the SPMD program has compile-time-fixed loop bounds).
  - Per layer: each core computes hs = dinv * (x W^T + b) for its own node
    shard (PE transpose + matmul), downcasts to bf16, and the shards are
    AllGathered into a full [C*NPAD, 128] bf16 table in DRAM.
  - Messages are fetched with batched indirect DMA gathers (one SWDGE
    instruction per ~hundred 128-edge tiles) and scatter-added per
    destination block with a one-hot matmul:
        agg_block[d, f] += S_tile[e, d]^T @ msg_tile[e, f]
    accumulated in PSUM. S_tile is built on the DVE with a single
    is_equal tensor_scalar of an iota row against per-edge dest ranks.
  - BN statistics (sum, sum of squares) are computed with mask-vector
    matmuls over the aggregated blocks and AllReduced across cores; the
    apply (scale/shift + relu + residual) runs on full-shard DVE/ACT ops.

kernel(**inputs) takes the FULL inputs and returns the FULL output.
"""

import numpy as np
import ml_dtypes

import concourse.bacc as bacc
import concourse.bass as bass
import concourse.mybir as mybir
import concourse.tile as tile
from concourse.bass_utils import run_bass_kernel_spmd
from concourse.masks import make_identity

P = 128
F32 = mybir.dt.float32
BF16 = mybir.dt.bfloat16
AF = mybir.ActivationFunctionType
ALU = mybir.AluOpType


class Cfg:
    def __init__(self, N, E, D, L, C, bpc, kg=4, bn_eps=1e-5):
        assert D == 128
        self.N, self.E, self.D, self.L, self.C = N, E, D, L, C
        self.NSH = N // C                      # real nodes per core
        assert self.NSH * C == N
        self.TPC = (self.NSH + P - 1) // P     # node tiles (blocks) per core
        self.NPAD = self.TPC * P               # padded nodes per core
        assert self.NSH < self.NPAD, "need at least one guaranteed-zero pad row"
        self.TROWS = C * self.NPAD             # gather table rows
        self.BPC = bpc                         # blocks per gather chunk
        self.chunks = [
            list(range(i, min(i + bpc, self.TPC))) for i in range(0, self.TPC, bpc)
        ]
        self.BN_EPS = bn_eps
        self.KG = kg  # max idxs per dma_gather call (in 128-edge tiles)
        self.ZROW = self.NSH  # core 0's first pad row: always written as zero
        self.LO = 32768
        if self.TROWS > self.LO:
            c_hi = -((self.LO - self.NSH) // -self.NPAD)
            zhi = c_hi * self.NPAD + self.NSH
            assert self.LO <= zhi < self.TROWS
            self.ZHI = zhi - self.LO
        else:
            self.ZHI = 0


def _preprocess(cfg, x, edge_index, W, b, gamma, beta):
    """All index/layout work on the host. Returns per-core in_maps and the
    (identical across cores) compile-time tile structure."""
    N, C, NSH, NPAD, TPC = cfg.N, cfg.C, cfg.NSH, cfg.NPAD, cfg.TPC
    row = np.asarray(edge_index[0], dtype=np.int64)
    col = np.asarray(edge_index[1], dtype=np.int64)
    x = np.asarray(x, dtype=np.float32)
    deg = np.bincount(row, minlength=N).astype(np.float32)  # out-degree
    deg_in = np.bincount(col, minlength=N)

    # Per-core local permutation: snake-deal destinations (sorted by
    # in-degree desc) into TPC blocks -> balanced per-block edge counts.
    newlocal = np.empty(N, np.int64)
    nblk0 = None
    for c in range(C):
        ids = np.arange(c * NSH, (c + 1) * NSH)
        order = ids[np.argsort(-deg_in[ids], kind="stable")]
        i = np.arange(NSH)
        r, j = i // TPC, i % TPC
        blk = np.where(r % 2 == 1, TPC - 1 - j, j)
        rank = np.zeros(NSH, np.int64)
        cnt = np.zeros(TPC, np.int64)
        for k in range(NSH):
            rank[k] = cnt[blk[k]]
            cnt[blk[k]] += 1
        newlocal[order] = blk * P + rank
        if nblk0 is None:
            nblk0 = cnt.copy()
        else:
            assert (cnt == nblk0).all()
    assert nblk0.max() <= P

    maskv = (np.arange(P)[:, None] < nblk0[None, :]).astype(np.float32)
    table_row = (np.arange(N) // NSH) * NPAD + newlocal  # node -> table row

    e_core = col // NSH
    e_blk = newlocal[col] // P
    e_rank = newlocal[col] % P
    e_src = table_row[row]

    # common tile structure: TLs/THs tiles per block, max over cores/blocks
    split_hi = cfg.TROWS > cfg.LO
    per = {}
    TLs, THs = 1, (1 if split_hi else 0)
    for c in range(C):
        selc = e_core == c
        for lo in (True, False):
            if not lo and not split_hi:
                continue
            sel = selc & ((e_src < cfg.LO) == lo)
            srcs, blks, ranks = e_src[sel], e_blk[sel], e_rank[sel]
            o = np.argsort(blks, kind="stable")
            srcs, blks, ranks = srcs[o], blks[o], ranks[o]
            starts = np.searchsorted(blks, np.arange(TPC))
            ends = np.searchsorted(blks, np.arange(TPC) + 1)
            per[(c, lo)] = (srcs, ranks, starts, ends)
            m = int((-((ends - starts) // -P)).max())
            if lo:
                TLs = max(TLs, m)
            else:
                THs = max(THs, m)
    if not split_hi:
        per = {(c, True): per[(c, True)] for c in range(C)}
    TS = TLs + THs
    NT = TPC * TS
    in_maps = []
    Wt = np.ascontiguousarray(np.transpose(np.asarray(W, np.float32), (0, 2, 1)))
    bT = np.ascontiguousarray(np.asarray(b, np.float32).T)

    def _wrap16(idx):
        w = idx.reshape(-1, 16).T.astype(np.int16)
        return np.ascontiguousarray(np.tile(w, (8, 1)))

    for c in range(C):
        idx_lo = np.full(TPC * TLs * P, cfg.ZROW, np.int64)
        idx_hi = np.full(max(TPC * THs * P, 16), cfg.ZHI, np.int64)
        # one-hot S matrices, block-contiguous: smat[e, (b*TS + t)*P + d]
        smat = np.zeros((P, NT * P), ml_dtypes.bfloat16)
        lo_off = hi_off = 0
        for ch in cfg.chunks:
            for bidx in ch:
                srcs, ranks, st, en = per[(c, True)]
                cnt = en[bidx] - st[bidx]
                idx_lo[lo_off : lo_off + cnt] = srcs[st[bidx]:en[bidx]]
                pos = np.arange(cnt)
                rr = ranks[st[bidx]:en[bidx]]
                smat[pos % P, (bidx * TS + pos // P) * P + rr] = 1.0
                lo_off += TLs * P
            for bidx in ch:
                if THs == 0:
                    continue
                srcs, ranks, st, en = per[(c, False)]
                cnt = en[bidx] - st[bidx]
                idx_hi[hi_off : hi_off + cnt] = srcs[st[bidx]:en[bidx]] - cfg.LO
                pos = np.arange(cnt)
                rr = ranks[st[bidx]:en[bidx]]
                smat[pos % P, (bidx * TS + TLs + pos // P) * P + rr] = 1.0
                hi_off += THs * P

        ids = np.arange(c * NSH, (c + 1) * NSH)
        xin = np.zeros((NPAD, cfg.D), np.float32)
        xin[newlocal[ids]] = x[ids]
        degT = np.zeros((P, TPC), np.float32)
        degT[newlocal[ids] % P, newlocal[ids] // P] = deg[ids]

        in_maps.append(
            {
                "xin": xin,
                "wt": Wt,
                "bT": bT,
                "gamma": np.asarray(gamma, np.float32),
                "beta": np.asarray(beta, np.float32),
                "degT": degT,
                "maskv": maskv,
                "smat": smat,
                "idx_lo": _wrap16(idx_lo),
                "idx_hi": _wrap16(idx_hi),
            }
        )

    meta = dict(TLs=TLs, THs=THs, NT=NT, newlocal=newlocal)
    return in_maps, meta


def _build(cfg, TLs, THs):
    """Build the SPMD Bass program (identical for all cores)."""
    N, D, L, C = cfg.N, cfg.D, cfg.L, cfg.C
    TPC, NPAD, TROWS = cfg.TPC, cfg.NPAD, cfg.TROWS
    TS = TLs + THs
    NT = TPC * TS
    NTC_MAX = max(len(ch) for ch in cfg.chunks) * TS

    nc = bacc.Bacc("TRN2", target_bir_lowering=False, debug=False, num_devices=C)

    xin = nc.dram_tensor("xin", [NPAD, D], F32, kind="ExternalInput")
    wt = nc.dram_tensor("wt", [L, D, D], F32, kind="ExternalInput")
    bT = nc.dram_tensor("bT", [D, L], F32, kind="ExternalInput")
    gamma_d = nc.dram_tensor("gamma", [L, D], F32, kind="ExternalInput")
    beta_d = nc.dram_tensor("beta", [L, D], F32, kind="ExternalInput")
    degT = nc.dram_tensor("degT", [P, TPC], F32, kind="ExternalInput")
    maskv_d = nc.dram_tensor("maskv", [P, TPC], F32, kind="ExternalInput")
    smat_d = nc.dram_tensor("smat", [P, NT * P], BF16, kind="ExternalInput")
    idx_lo_d = nc.dram_tensor(
        "idx_lo", [P, TPC * TLs * P // 16], mybir.dt.int16, kind="ExternalInput"
    )
    nhi16 = max(TPC * THs * P, 16) // 16
    idx_hi_d = nc.dram_tensor(
        "idx_hi", [P, nhi16], mybir.dt.int16, kind="ExternalInput"
    )
    out_d = nc.dram_tensor("out", [NPAD, D], F32, kind="ExternalOutput")

    rg = [list(range(C))]

    with tile.TileContext(nc) as tc:
        with (
            tc.tile_pool(name="persist", bufs=1) as pp,
            tc.tile_pool(name="msgp", bufs=max(2, 112 // cfg.KG)) as msgp,
            tc.tile_pool(name="bigtmp", bufs=1) as btp,
            tc.tile_pool(name="sp", bufs=4) as sp,
            tc.tile_pool(name="work", bufs=4) as wp,
            tc.tile_pool(name="psblk", bufs=2, space="PSUM") as psblk,
            tc.tile_pool(name="psmisc", bufs=3, space="PSUM") as psmisc,
            tc.tile_pool(name="psbc", bufs=1, space="PSUM") as psbc,
            tc.tile_pool(name="psstat", bufs=2, space="PSUM") as psstat,
            tc.tile_pool(name="dram", bufs=1, space="DRAM") as dp,
        ):
            # ---- persistent loads ----
            x_sb = pp.tile([P, TPC, D], F32)
            nc.sync.dma_start(x_sb[:], xin[:].rearrange("(t p) f -> p t f", p=P))
            wt_sb = pp.tile([P, L, D], F32)
            for l in range(L):
                nc.sync.dma_start(wt_sb[:, l, :], wt[l, :, :])
            bT_sb = pp.tile([P, L], F32)
            nc.sync.dma_start(bT_sb[:], bT[:])
            gb_sb = pp.tile([1, 2 * L, D], F32)  # gamma/beta rows on partition 0
            for l in range(L):
                nc.sync.dma_start(gb_sb[:, l, :], gamma_d[l : l + 1, :])
                nc.sync.dma_start(gb_sb[:, L + l, :], beta_d[l : l + 1, :])
            deg_sb = pp.tile([P, TPC], F32)
            nc.sync.dma_start(deg_sb[:], degT[:])
            maskv_sb = pp.tile([P, TPC], F32)
            nc.sync.dma_start(maskv_sb[:], maskv_d[:])
            idx_lo_sb = pp.tile([P, TPC * TLs * P // 16], mybir.dt.int16)
            nc.sync.dma_start(idx_lo_sb[:], idx_lo_d[:])
            idx_hi_sb = pp.tile([P, nhi16], mybir.dt.int16)
            nc.sync.dma_start(idx_hi_sb[:], idx_hi_d[:])
            ident = pp.tile([P, P], F32)
            make_identity(nc, ident[:])
            ones1 = pp.tile([1, P], F32)
            nc.vector.memset(ones1[:], 1.0)

            # dinv = (deg > 0) / sqrt(max(deg, 1))
            dinv_sb = pp.tile([P, TPC], F32)
            t_a = wp.tile([P, TPC], F32, tag="dinv")
            nc.vector.tensor_scalar_max(t_a[:], deg_sb[:], 1.0)
            t_b = wp.tile([P, TPC], F32, tag="dinv")
            nc.vector.reciprocal(t_b[:], t_a[:])
            t_c = wp.tile([P, TPC], F32, tag="dinv")
            nc.scalar.sqrt(t_c[:], t_b[:])
            t_d = wp.tile([P, TPC], F32, tag="dinv")
            nc.vector.tensor_scalar(t_d[:], deg_sb[:], 0.0, None, ALU.is_gt)
            nc.vector.tensor_tensor(dinv_sb[:], t_c[:], t_d[:], ALU.mult)

            agg_sb = pp.tile([P, TPC, D], F32)
            hs_sb = pp.tile([P, TPC, D], BF16)

            # DRAM collective buffers
            shard_dr = dp.tile([NPAD, D], BF16)
            table_dr = nc.dram_tensor(
                "table_sh", [TROWS, D], BF16, kind="Internal", addr_space="Shared"
            )
            stats_in = dp.tile([1, 2 * D], F32)
            stats_out = dp.tile([1, 2 * D], F32)

            for l in range(L):
                # ---- hs = dinv * (x @ W^T + b), downcast bf16, row-major ----
                for t in range(TPC):
                    xT_ps = psmisc.tile([P, P], F32, tag="ps")
                    nc.tensor.transpose(xT_ps[:], x_sb[:, t, :], ident[:])
                    xT = wp.tile([P, P], F32, tag="xT")
                    nc.vector.tensor_copy(xT[:], xT_ps[:])
                    hT_ps = psmisc.tile([P, P], F32, tag="ps")
                    nc.tensor.matmul(
                        out=hT_ps[:], lhsT=wt_sb[:, l, :], rhs=xT[:],
                        start=True, stop=True,
                    )
                    hb = wp.tile([P, P], F32, tag="hb")
                    nc.scalar.activation(
                        hb[:], hT_ps[:], AF.Identity, bias=bT_sb[:, l : l + 1]
                    )
                    h_rm_ps = psmisc.tile([P, P], F32, tag="ps")
                    nc.tensor.transpose(h_rm_ps[:], hb[:], ident[:])
                    nc.scalar.activation(
                        hs_sb[:, t, :], h_rm_ps[:], AF.Identity,
                        scale=dinv_sb[:, t : t + 1],
                    )
                nc.sync.dma_start(
                    shard_dr[:].rearrange("(t p) f -> p t f", p=P), hs_sb[:]
                )
                nc.gpsimd.collective_compute(
                    "AllGather",
                    ALU.bypass,
                    ins=[shard_dr.opt()],
                    outs=[table_dr[:].opt()],
                    replica_groups=rg,
                )

                # ---- gather + one-hot matmul aggregation ----
                stA_ps = psstat.tile([1, P], F32, tag="st")
                stB_ps = psstat.tile([1, P], F32, tag="st")
                tile_col = 0
                lo_off = hi_off = 0
                for ch in cfg.chunks:
                    nb = len(ch)
                    ntc = nb * TS
                    # one msg tile per gather call (KG tiles each) for deep
                    # DMA pipelining via the pool; slot_of maps a chunk-local
                    # msg-tile column to its (pool tile, slot)
                    slot_of = {}

                    def _mt(mcol, _s=None):
                        mt, sl = slot_of[mcol]
                        return mt[:, sl, :]

                    nlo = nb * TLs * P
                    KGP = cfg.KG * P
                    for g0 in range(0, nlo, KGP):
                        g1 = min(g0 + KGP, nlo)
                        mt = msgp.tile([P, cfg.KG, D], BF16, tag="msg")
                        for i in range((g1 - g0) // P):
                            slot_of[g0 // P + i] = (mt, i)
                        nc.gpsimd.dma_gather(
                            mt[:, : (g1 - g0) // P, :],
                            table_dr[:],
                            idx_lo_sb[:, (lo_off + g0) // 16 : (lo_off + g1) // 16],
                            g1 - g0, g1 - g0, D,
                        )
                    lo_off += nlo
                    if THs > 0:
                        nhi = nb * THs * P
                        for g0 in range(0, nhi, KGP):
                            g1 = min(g0 + KGP, nhi)
                            mt = msgp.tile([P, cfg.KG, D], BF16, tag="msg")
                            for i in range((g1 - g0) // P):
                                slot_of[nb * TLs + g0 // P + i] = (mt, i)
                            nc.gpsimd.dma_gather(
                                mt[:, : (g1 - g0) // P, :],
                                table_dr[cfg.LO :, :],
                                idx_hi_sb[
                                    :, (hi_off + g0) // 16 : (hi_off + g1) // 16
                                ],
                                g1 - g0, g1 - g0, D,
                            )
                        hi_off += nhi
                    for j, bidx in enumerate(ch):
                        ps_b = psblk.tile([P, P], F32, tag="blk")
                        s_blk = sp.tile([P, TS, P], BF16, tag="s")
                        nc.sync.dma_start(
                            s_blk[:],
                            smat_d[:, bidx * TS * P : (bidx + 1) * TS * P],
                        )
                        mm, nmm = 0, TS
                        for t in range(TLs):
                            mcol = j * TLs + t
                            nc.tensor.matmul(
                                out=ps_b[:], lhsT=s_blk[:, t, :], rhs=_mt(mcol),
                                start=(mm == 0), stop=(mm == nmm - 1),
                            )
                            mm += 1
                        for t in range(THs):
                            mcol = nb * TLs + j * THs + t
                            nc.tensor.matmul(
                                out=ps_b[:], lhsT=s_blk[:, TLs + t, :], rhs=_mt(mcol),
                                start=(mm == 0), stop=(mm == nmm - 1),
                            )
                            mm += 1
                        nc.scalar.activation(
                            agg_sb[:, bidx, :], ps_b[:], AF.Identity,
                            scale=dinv_sb[:, bidx : bidx + 1],
                        )
                        nc.tensor.matmul(
                            out=stA_ps[:],
                            lhsT=maskv_sb[:, bidx : bidx + 1],
                            rhs=agg_sb[:, bidx, :],
                            start=(bidx == 0), stop=(bidx == TPC - 1),
                            skip_group_check=True,
                        )
                        aggsq = wp.tile([P, P], F32, tag="aggsq")
                        nc.scalar.square(aggsq[:], agg_sb[:, bidx, :])
                        nc.tensor.matmul(
                            out=stB_ps[:],
                            lhsT=maskv_sb[:, bidx : bidx + 1],
                            rhs=aggsq[:],
                            start=(bidx == 0), stop=(bidx == TPC - 1),
                            skip_group_check=True,
                        )
                    tile_col += ntc

                st_sb = wp.tile([1, 2, P], F32, tag="st")
                nc.vector.tensor_copy(st_sb[:, 0, :], stA_ps[:])
                nc.vector.tensor_copy(st_sb[:, 1, :], stB_ps[:])
                nc.sync.dma_start(stats_in[:], st_sb[:])
                nc.gpsimd.collective_compute(
                    "AllReduce",
                    ALU.add,
                    ins=[stats_in.opt()],
                    outs=[stats_out.opt()],
                    replica_groups=rg,
                )
                stg = wp.tile([1, 2, P], F32, tag="st")
                nc.sync.dma_start(stg[:], stats_out[:])

                # ---- scale/shift vectors on partition 0 ----
                vec = wp.tile([1, 8, P], F32, tag="vec")
                MU, MSQ, VAR, RSTD, SC, SH, T0, T1 = range(8)
                inv_n = 1.0 / float(N)
                nc.vector.tensor_scalar_mul(vec[:, MU, :], stg[:, 0, :], inv_n)
                nc.vector.tensor_scalar_mul(vec[:, MSQ, :], stg[:, 1, :], inv_n)
                nc.vector.tensor_tensor(
                    vec[:, T0, :], vec[:, MU, :], vec[:, MU, :], ALU.mult
                )
                nc.vector.tensor_tensor(
                    vec[:, VAR, :], vec[:, MSQ, :], vec[:, T0, :], ALU.subtract
                )
                nc.vector.tensor_scalar_add(vec[:, T1, :], vec[:, VAR, :], cfg.BN_EPS)
                nc.vector.reciprocal(vec[:, T0, :], vec[:, T1, :])
                nc.scalar.sqrt(vec[:, RSTD, :], vec[:, T0, :])
                nc.vector.tensor_tensor(
                    vec[:, SC, :], gb_sb[:, l, :], vec[:, RSTD, :], ALU.mult
                )
                nc.vector.tensor_tensor(
                    vec[:, T0, :], vec[:, MU, :], vec[:, SC, :], ALU.mult
                )
                nc.vector.tensor_tensor(
                    vec[:, SH, :], gb_sb[:, L + l, :], vec[:, T0, :], ALU.subtract
                )
                # broadcast scale|shift across partitions via ones-matmul
                bc_ps = psbc.tile([P, 2 * P], F32, tag="bc")
                nc.tensor.matmul(
                    out=bc_ps[:], lhsT=ones1[:], rhs=vec[:, SC : SH + 1, :],
                    start=True, stop=True,
                )
                screp = wp.tile([P, 2, P], F32, tag="screp")
                nc.vector.tensor_copy(screp[:], bc_ps[:])

                # ---- BN apply + relu + residual (whole shard) ----
                t1 = btp.tile([P, TPC, D], F32, tag="t1")
                nc.vector.tensor_tensor(
                    t1[:],
                    agg_sb[:],
                    screp[:, 0:1, :].to_broadcast([P, TPC, D]),
                    ALU.mult,
                )
                nc.vector.tensor_tensor(
                    t1[:],
                    t1[:],
                    screp[:, 1:2, :].to_broadcast([P, TPC, D]),
                    ALU.add,
                )
                nc.scalar.activation(t1[:], t1[:], AF.Relu)
                nc.vector.tensor_tensor(x_sb[:], x_sb[:], t1[:], ALU.add)

            nc.sync.dma_start(out_d[:].rearrange("(t p) f -> p t f", p=P), x_sb[:])

    nc.compile()
    return nc


_CACHE = {}


def _get_nc(cfg, TLs, THs):
    key = (cfg.N, cfg.E, cfg.L, cfg.C, cfg.BPC, cfg.KG, TLs, THs)
    if key not in _CACHE:
        _CACHE[key] = _build(cfg, TLs, THs)
    return _CACHE[key]


def run(cfg, inputs, trace=False):
    in_maps, meta = _preprocess(cfg, **inputs)
    nc = _get_nc(cfg, meta["TLs"], meta["THs"])
    res = run_bass_kernel_spmd(nc, in_maps, core_ids=list(range(cfg.C)), trace=trace)
    newlocal = meta["newlocal"]
    xfull = np.empty((cfg.N, cfg.D), np.float32)
    for c in range(cfg.C):
        ids = np.arange(c * cfg.NSH, (c + 1) * cfg.NSH)
        xfull[ids] = res.results[c]["out"][newlocal[ids]]
    return xfull, res


def kernel(x, edge_index, W, b, gamma, beta):
    cfg = Cfg(N=50000, E=800000, D=128, L=3, C=8, bpc=7, kg=8)
    out, _ = run(
        cfg, dict(x=x, edge_index=edge_index, W=W, b=b, gamma=gamma, beta=beta)
    )
    return out



# revision 10
# speedup vs baseline: 1.1450x; 1.0570x over previous
"""GCN message-passing kernel for 8 Trainium2 NeuronCores.

Strategy (graph/data parallel, per the sharding hint):
  - Destination nodes are sharded across the 8 cores in contiguous ranges.
  - Within each core, its destinations are dealt (by in-degree, snake order)
    into 128-wide blocks so per-block edge counts are balanced across
    blocks AND cores (the SPMD program has compile-time-fixed loop bounds).
  - Per layer: each core computes hs = dinv * (x W^T + b) for its own node
    shard, downcasts to bf16. The shard is split in two halves (local rows
    [0,L1) and [L1,NPAD)); each half is AllGathered into its own Shared-HBM
    table (T1: C*L1 rows, T2: C*(NPAD-L1) rows; both < 32768 so int16
    gather indices cover them without a hi/lo base split).
  - The gpsimd instruction stream interleaves the two collectives with the
    gather calls so SWDGE descriptor generation (the bottleneck engine)
    starts as soon as T1 is ready and never waits for T2:
        AG-A, G1(c0), G1(c1), AG-B, G2(c0), MM(c0), G1(c2), G2(c1), ...
  - Messages are fetched with batched indirect DMA gathers and scatter-added
    per destination block with one-hot matmuls accumulated in PSUM:
        agg_block[d, f] += S_tile[e, d]^T @ msg_tile[e, f]
  - BN statistics (sum, sum of squares) are computed with mask-vector
    matmuls over the aggregated blocks and AllReduced across cores; the
    apply (scale/shift + relu + residual) runs on full-shard DVE/ACT ops.

kernel(**inputs) takes the FULL inputs and returns the FULL output.
"""

import numpy as np
import ml_dtypes

import concourse.bacc as bacc
import concourse.bass as bass
import concourse.mybir as mybir
import concourse.tile as tile
from concourse.bass_utils import run_bass_kernel_spmd
from concourse.masks import make_identity

P = 128
F32 = mybir.dt.float32
BF16 = mybir.dt.bfloat16
AF = mybir.ActivationFunctionType
ALU = mybir.AluOpType


class Cfg:
    def __init__(self, N, E, D, L, C, bpc, kg=8, bn_eps=1e-5, l1=3968):
        assert D == 128
        self.N, self.E, self.D, self.L, self.C = N, E, D, L, C
        self.NSH = N // C                      # real nodes per core
        assert self.NSH * C == N
        self.TPC = (self.NSH + P - 1) // P     # node tiles (blocks) per core
        self.NPAD = self.TPC * P               # padded nodes per core
        assert self.NSH < self.NPAD, "need at least one guaranteed-zero pad row"
        self.L1 = l1                           # local-row split point (A half)
        assert l1 % P == 0 and 0 < l1 < self.NPAD
        self.TA = l1 // P                      # A-half blocks
        assert C * l1 <= 32768 and C * (self.NPAD - l1) <= 32768
        self.TR1 = C * l1                      # T1 table rows
        self.TR2 = C * (self.NPAD - l1)        # T2 table rows
        self.BPC = bpc                         # blocks per chunk
        self.chunks = [
            list(range(i, min(i + bpc, self.TPC))) for i in range(0, self.TPC, bpc)
        ]
        self.BN_EPS = bn_eps
        self.KG = kg  # max idxs per dma_gather call (in 128-edge tiles)


def _preprocess(cfg, x, edge_index, W, b, gamma, beta):
    """All index/layout work on the host. Returns per-core in_maps and the
    (identical across cores) compile-time tile structure."""
    N, C, NSH, NPAD, TPC = cfg.N, cfg.C, cfg.NSH, cfg.NPAD, cfg.TPC
    L1, TA = cfg.L1, cfg.TA
    row = np.asarray(edge_index[0], dtype=np.int64)
    col = np.asarray(edge_index[1], dtype=np.int64)
    x = np.asarray(x, dtype=np.float32)
    deg = np.bincount(row, minlength=N).astype(np.float32)  # out-degree
    deg_in = np.bincount(col, minlength=N)

    # Pass 1: snake-deal destinations (sorted by in-degree desc) into TPC
    # blocks; this only FIXES each node's A/B half (block < TA -> half A,
    # i.e. the node's hs row lands in table T1). Halves then determine the
    # per-node split of in-edges by source table (w1/w2), which pass 2
    # balances per block under hard tile caps.
    last_r = (NSH - 1) // TPC
    halfb = np.empty(N, np.int64)
    for c in range(C):
        ids = np.arange(c * NSH, (c + 1) * NSH)
        order = ids[np.argsort(-deg_in[ids], kind="stable")]
        i = np.arange(NSH)
        r, j = i // TPC, i % TPC
        blk = np.where((r % 2 == 1) & (r != last_r), TPC - 1 - j, j)
        halfb[order] = (blk >= TA).astype(np.int64)
    w1 = np.bincount(col[halfb[row] == 0], minlength=N).astype(np.int64)
    w2 = np.bincount(col[halfb[row] == 1], minlength=N).astype(np.int64)

    # Pass 2: within each half, greedy 2D deal balancing (w1, w2) per
    # block. Rank 127 of blocks TA-1 and TPC-1 stays empty: guaranteed
    # all-zero gather targets for index padding in each table.
    newlocal = np.empty(N, np.int64)
    cap1, cap2 = 11 * P, 6 * P
    maskvs = []
    for c in range(C):
        ids = np.arange(c * NSH, (c + 1) * NSH)
        cnt = np.zeros(TPC, np.int64)
        l1b = np.zeros(TPC, np.int64)
        l2b = np.zeros(TPC, np.int64)
        capn = np.full(TPC, P, np.int64)
        capn[TA - 1] = P - 1
        capn[TPC - 1] = P - 1
        for h, blo, bhi in ((0, 0, TA), (1, TA, TPC)):
            sel = ids[halfb[ids] == h]
            o = np.argsort(-(w1[sel] + w2[sel]), kind="stable")
            bb = np.arange(blo, bhi)
            for v in sel[o]:
                open_ = bb[cnt[bb] < capn[bb]]
                m = np.maximum(
                    (l1b[open_] + w1[v]) * cap2, (l2b[open_] + w2[v]) * cap1
                )
                best = open_[np.argmin(m)]
                newlocal[v] = best * P + cnt[best]
                cnt[best] += 1
                l1b[best] += w1[v]
                l2b[best] += w2[v]
        maskvs.append(
            (np.arange(P)[:, None] < cnt[None, :]).astype(np.float32)
        )
    Z1 = (TA - 1) * P + P - 1           # local row, < L1, always zero
    Z2 = (TPC - 1) * P + P - 1 - L1     # T2-local row, always zero

    src_local = newlocal[row]
    src_core = row // NSH
    e_core = col // NSH
    e_blk = newlocal[col] // P
    e_rank = newlocal[col] % P
    in_t1 = src_local < L1
    idx1_full = src_core * L1 + src_local            # valid where in_t1
    idx2_full = src_core * (NPAD - L1) + (src_local - L1)

    # common tile structure: T1s/T2s tiles per block, max over cores/blocks
    per = {}
    T1s, T2s = 1, 1
    for c in range(C):
        selc = e_core == c
        for first in (True, False):
            sel = selc & (in_t1 == first)
            srcs = (idx1_full if first else idx2_full)[sel]
            blks, ranks = e_blk[sel], e_rank[sel]
            o = np.argsort(blks, kind="stable")
            srcs, blks, ranks = srcs[o], blks[o], ranks[o]
            starts = np.searchsorted(blks, np.arange(TPC))
            ends = np.searchsorted(blks, np.arange(TPC) + 1)
            per[(c, first)] = (srcs, ranks, starts, ends)
            m = int((-((ends - starts) // -P)).max())
            if first:
                T1s = max(T1s, m)
            else:
                T2s = max(T2s, m)
    TS = T1s + T2s
    NT = TPC * TS
    in_maps = []
    Wt = np.ascontiguousarray(np.transpose(np.asarray(W, np.float32), (0, 2, 1)))
    bT = np.ascontiguousarray(np.asarray(b, np.float32).T)

    def _wrap16(idx):
        w = idx.reshape(-1, 16).T.astype(np.int16)
        return np.ascontiguousarray(np.tile(w, (8, 1)))

    for c in range(C):
        idx_1 = np.full(TPC * T1s * P, Z1, np.int64)
        idx_2 = np.full(TPC * T2s * P, Z2, np.int64)
        # one-hot S matrices, block-contiguous: smat[e, (b*TS + t)*P + d]
        smat = np.zeros((P, NT * P), ml_dtypes.bfloat16)
        o1 = o2 = 0
        for ch in cfg.chunks:
            for bidx in ch:
                srcs, ranks, st, en = per[(c, True)]
                cnt = en[bidx] - st[bidx]
                idx_1[o1 : o1 + cnt] = srcs[st[bidx]:en[bidx]]
                pos = np.arange(cnt)
                rr = ranks[st[bidx]:en[bidx]]
                smat[pos % P, (bidx * TS + pos // P) * P + rr] = 1.0
                o1 += T1s * P
            for bidx in ch:
                srcs, ranks, st, en = per[(c, False)]
                cnt = en[bidx] - st[bidx]
                idx_2[o2 : o2 + cnt] = srcs[st[bidx]:en[bidx]]
                pos = np.arange(cnt)
                rr = ranks[st[bidx]:en[bidx]]
                smat[pos % P, (bidx * TS + T1s + pos // P) * P + rr] = 1.0
                o2 += T2s * P

        ids = np.arange(c * NSH, (c + 1) * NSH)
        xin = np.zeros((NPAD, cfg.D), np.float32)
        xin[newlocal[ids]] = x[ids]
        degT = np.zeros((P, TPC), np.float32)
        degT[newlocal[ids] % P, newlocal[ids] // P] = deg[ids]

        in_maps.append(
            {
                "xin": xin,
                "wt": Wt,
                "bT": bT,
                "gamma": np.asarray(gamma, np.float32),
                "beta": np.asarray(beta, np.float32),
                "degT": degT,
                "maskv": maskvs[c],
                "smat": smat,
                "idx1": _wrap16(idx_1),
                "idx2": _wrap16(idx_2),
            }
        )

    meta = dict(T1s=T1s, T2s=T2s, NT=NT, newlocal=newlocal)
    return in_maps, meta


def _build(cfg, T1s, T2s):
    """Build the SPMD Bass program (identical for all cores)."""
    N, D, L, C = cfg.N, cfg.D, cfg.L, cfg.C
    TPC, NPAD, L1, TA = cfg.TPC, cfg.NPAD, cfg.L1, cfg.TA
    TS = T1s + T2s
    NT = TPC * TS

    nc = bacc.Bacc("TRN2", target_bir_lowering=False, debug=False, num_devices=C)

    xin = nc.dram_tensor("xin", [NPAD, D], F32, kind="ExternalInput")
    wt = nc.dram_tensor("wt", [L, D, D], F32, kind="ExternalInput")
    bT = nc.dram_tensor("bT", [D, L], F32, kind="ExternalInput")
    gamma_d = nc.dram_tensor("gamma", [L, D], F32, kind="ExternalInput")
    beta_d = nc.dram_tensor("beta", [L, D], F32, kind="ExternalInput")
    degT = nc.dram_tensor("degT", [P, TPC], F32, kind="ExternalInput")
    maskv_d = nc.dram_tensor("maskv", [P, TPC], F32, kind="ExternalInput")
    smat_d = nc.dram_tensor("smat", [P, NT * P], BF16, kind="ExternalInput")
    idx1_d = nc.dram_tensor(
        "idx1", [P, TPC * T1s * P // 16], mybir.dt.int16, kind="ExternalInput"
    )
    idx2_d = nc.dram_tensor(
        "idx2", [P, TPC * T2s * P // 16], mybir.dt.int16, kind="ExternalInput"
    )
    out_d = nc.dram_tensor("out", [NPAD, D], F32, kind="ExternalOutput")

    rg = [list(range(C))]

    with tile.TileContext(nc) as tc:
        with (
            tc.tile_pool(name="persist", bufs=1) as pp,
            tc.tile_pool(name="msgp", bufs=32) as msgp,
            tc.tile_pool(name="sp", bufs=4) as sp,
            tc.tile_pool(name="work", bufs=4) as wp,
            tc.tile_pool(name="psblk", bufs=2, space="PSUM") as psblk,
            tc.tile_pool(name="psmisc", bufs=3, space="PSUM") as psmisc,
            tc.tile_pool(name="psbc", bufs=1, space="PSUM") as psbc,
            tc.tile_pool(name="psstat", bufs=2, space="PSUM") as psstat,
            tc.tile_pool(name="dram", bufs=1, space="DRAM") as dp,
        ):
            # ---- persistent loads ----
            x_sb = pp.tile([P, TPC, D], F32)
            nc.sync.dma_start(x_sb[:], xin[:].rearrange("(t p) f -> p t f", p=P))
            wt_sb = pp.tile([P, L, D], F32)
            for l in range(L):
                nc.sync.dma_start(wt_sb[:, l, :], wt[l, :, :])
            bT_sb = pp.tile([P, L], F32)
            nc.sync.dma_start(bT_sb[:], bT[:])
            gb_sb = pp.tile([1, 2 * L, D], F32)  # gamma/beta rows on partition 0
            for l in range(L):
                nc.sync.dma_start(gb_sb[:, l, :], gamma_d[l : l + 1, :])
                nc.sync.dma_start(gb_sb[:, L + l, :], beta_d[l : l + 1, :])
            deg_sb = pp.tile([P, TPC], F32)
            nc.sync.dma_start(deg_sb[:], degT[:])
            maskv_sb = pp.tile([P, TPC], F32)
            nc.sync.dma_start(maskv_sb[:], maskv_d[:])
            idx1_sb = pp.tile([P, TPC * T1s * P // 16], mybir.dt.int16)
            nc.sync.dma_start(idx1_sb[:], idx1_d[:])
            idx2_sb = pp.tile([P, TPC * T2s * P // 16], mybir.dt.int16)
            nc.sync.dma_start(idx2_sb[:], idx2_d[:])
            ident = pp.tile([P, P], F32)
            make_identity(nc, ident[:])
            ones1 = pp.tile([1, P], F32)
            nc.vector.memset(ones1[:], 1.0)

            # dinv = (deg > 0) / sqrt(max(deg, 1))
            dinv_sb = pp.tile([P, TPC], F32)
            t_a = wp.tile([P, TPC], F32, tag="dinv")
            nc.vector.tensor_scalar_max(t_a[:], deg_sb[:], 1.0)
            t_b = wp.tile([P, TPC], F32, tag="dinv")
            nc.vector.reciprocal(t_b[:], t_a[:])
            t_c = wp.tile([P, TPC], F32, tag="dinv")
            nc.scalar.sqrt(t_c[:], t_b[:])
            t_d = wp.tile([P, TPC], F32, tag="dinv")
            nc.vector.tensor_scalar(t_d[:], deg_sb[:], 0.0, None, ALU.is_gt)
            nc.vector.tensor_tensor(dinv_sb[:], t_c[:], t_d[:], ALU.mult)

            agg_sb = pp.tile([P, TPC, D], F32)
            hs_sb = pp.tile([P, TPC, D], BF16)

            # DRAM buffers: local shard halves, Shared gather tables
            shardA_dr = dp.tile([L1, D], BF16)
            shardB_dr = dp.tile([NPAD - L1, D], BF16)
            table1 = nc.dram_tensor(
                "table1_sh", [cfg.TR1, D], BF16, kind="Internal", addr_space="Shared"
            )
            table2 = nc.dram_tensor(
                "table2_sh", [cfg.TR2, D], BF16, kind="Internal", addr_space="Shared"
            )
            stats_in = dp.tile([1, 2 * D], F32)
            stats_out = dp.tile([1, 2 * D], F32)

            def _hs_tiles(l, t0, t1):
                for t in range(t0, t1):
                    xT_ps = psmisc.tile([P, P], F32, tag="ps")
                    nc.tensor.transpose(xT_ps[:], x_sb[:, t, :], ident[:])
                    xT = wp.tile([P, P], F32, tag="xT")
                    nc.vector.tensor_copy(xT[:], xT_ps[:])
                    hT_ps = psmisc.tile([P, P], F32, tag="ps")
                    nc.tensor.matmul(
                        out=hT_ps[:], lhsT=wt_sb[:, l, :], rhs=xT[:],
                        start=True, stop=True,
                    )
                    hb = wp.tile([P, P], F32, tag="hb")
                    nc.scalar.activation(
                        hb[:], hT_ps[:], AF.Identity, bias=bT_sb[:, l : l + 1]
                    )
                    h_rm_ps = psmisc.tile([P, P], F32, tag="ps")
                    nc.tensor.transpose(h_rm_ps[:], hb[:], ident[:])
                    nc.scalar.activation(
                        hs_sb[:, t, :], h_rm_ps[:], AF.Identity,
                        scale=dinv_sb[:, t : t + 1],
                    )

            for l in range(L):
                # ---- hs halves + AllGathers into Shared tables ----
                _hs_tiles(l, 0, TA)
                nc.sync.dma_start(
                    shardA_dr[:].rearrange("(t p) f -> p t f", p=P),
                    hs_sb[:, 0:TA, :],
                )
                nc.gpsimd.collective_compute(
                    "AllGather",
                    ALU.bypass,
                    ins=[shardA_dr.opt()],
                    outs=[table1[:].opt()],
                    replica_groups=rg,
                )
                _hs_tiles(l, TA, TPC)
                nc.sync.dma_start(
                    shardB_dr[:].rearrange("(t p) f -> p t f", p=P),
                    hs_sb[:, TA:TPC, :],
                )

                # ---- interleaved gathers + per-chunk one-hot matmuls ----
                stA_ps = psstat.tile([1, P], F32, tag="st")
                stB_ps = psstat.tile([1, P], F32, tag="st")
                KGP = cfg.KG * P
                o1 = o2 = 0
                chunk_slots = {}
                nchunks = len(cfg.chunks)

                def _gather(k, first):
                    nonlocal o1, o2
                    nb = len(cfg.chunks[k])
                    slots = chunk_slots.setdefault(k, {})
                    if first:
                        nrow, off, base, idx_sb, tbl = (
                            nb * T1s * P, 0, o1, idx1_sb, table1,
                        )
                    else:
                        nrow, off, base, idx_sb, tbl = (
                            nb * T2s * P, nb * T1s, o2, idx2_sb, table2,
                        )
                    for g0 in range(0, nrow, KGP):
                        g1 = min(g0 + KGP, nrow)
                        mt = msgp.tile([P, cfg.KG, D], BF16, tag="msg")
                        for i in range((g1 - g0) // P):
                            slots[off + g0 // P + i] = (mt, i)
                        nc.gpsimd.dma_gather(
                            mt[:, : (g1 - g0) // P, :],
                            tbl[:],
                            idx_sb[:, (base + g0) // 16 : (base + g1) // 16],
                            g1 - g0, g1 - g0, D,
                        )
                    if first:
                        o1 += nrow
                    else:
                        o2 += nrow

                def _mms(k):
                    nb = len(cfg.chunks[k])
                    slots = chunk_slots.pop(k)
                    for j, bidx in enumerate(cfg.chunks[k]):
                        ps_b = psblk.tile([P, P], F32, tag="blk")
                        s_blk = sp.tile([P, TS, P], BF16, tag="s")
                        nc.sync.dma_start(
                            s_blk[:],
                            smat_d[:, bidx * TS * P : (bidx + 1) * TS * P],
                        )
                        for t in range(TS):
                            if t < T1s:
                                mcol = j * T1s + t
                            else:
                                mcol = nb * T1s + j * T2s + (t - T1s)
                            mt, sl = slots[mcol]
                            nc.tensor.matmul(
                                out=ps_b[:], lhsT=s_blk[:, t, :], rhs=mt[:, sl, :],
                                start=(t == 0), stop=(t == TS - 1),
                            )
                        nc.scalar.activation(
                            agg_sb[:, bidx, :], ps_b[:], AF.Identity,
                            scale=dinv_sb[:, bidx : bidx + 1],
                        )
                        nc.tensor.matmul(
                            out=stA_ps[:],
                            lhsT=maskv_sb[:, bidx : bidx + 1],
                            rhs=agg_sb[:, bidx, :],
                            start=(bidx == 0), stop=(bidx == TPC - 1),
                            skip_group_check=True,
                        )
                        aggsq = wp.tile([P, P], F32, tag="aggsq")
                        nc.scalar.square(aggsq[:], agg_sb[:, bidx, :])
                        nc.tensor.matmul(
                            out=stB_ps[:],
                            lhsT=maskv_sb[:, bidx : bidx + 1],
                            rhs=aggsq[:],
                            start=(bidx == 0), stop=(bidx == TPC - 1),
                            skip_group_check=True,
                        )

                # schedule: G1(0) G1(1) AGB G2(0) MM(0) G1(2) G2(1) MM(1) ...
                # (lag-1: G1(k) tiles live until MM(k), a <=32-buf window)
                _gather(0, True)
                for k in range(nchunks):
                    if k + 1 < nchunks:
                        _gather(k + 1, True)
                    if k == 0:
                        nc.gpsimd.collective_compute(
                            "AllGather",
                            ALU.bypass,
                            ins=[shardB_dr.opt()],
                            outs=[table2[:].opt()],
                            replica_groups=rg,
                        )
                    _gather(k, False)
                    _mms(k)

                st_sb = wp.tile([1, 2, P], F32, tag="st")
                nc.vector.tensor_copy(st_sb[:, 0, :], stA_ps[:])
                nc.vector.tensor_copy(st_sb[:, 1, :], stB_ps[:])
                nc.sync.dma_start(stats_in[:], st_sb[:])
                nc.gpsimd.collective_compute(
                    "AllReduce",
                    ALU.add,
                    ins=[stats_in.opt()],
                    outs=[stats_out.opt()],
                    replica_groups=rg,
                )
                stg = wp.tile([1, 2, P], F32, tag="st")
                nc.sync.dma_start(stg[:], stats_out[:])

                # ---- scale/shift vectors on partition 0 ----
                vec = wp.tile([1, 8, P], F32, tag="vec")
                MU, MSQ, VAR, RSTD, SC, SH, T0, T1 = range(8)
                inv_n = 1.0 / float(N)
                nc.vector.tensor_scalar_mul(vec[:, MU, :], stg[:, 0, :], inv_n)
                nc.vector.tensor_scalar_mul(vec[:, MSQ, :], stg[:, 1, :], inv_n)
                nc.vector.tensor_tensor(
                    vec[:, T0, :], vec[:, MU, :], vec[:, MU, :], ALU.mult
                )
                nc.vector.tensor_tensor(
                    vec[:, VAR, :], vec[:, MSQ, :], vec[:, T0, :], ALU.subtract
                )
                nc.vector.tensor_scalar_add(vec[:, T1, :], vec[:, VAR, :], cfg.BN_EPS)
                nc.vector.reciprocal(vec[:, T0, :], vec[:, T1, :])
                nc.scalar.sqrt(vec[:, RSTD, :], vec[:, T0, :])
                nc.vector.tensor_tensor(
                    vec[:, SC, :], gb_sb[:, l, :], vec[:, RSTD, :], ALU.mult
                )
                nc.vector.tensor_tensor(
                    vec[:, T0, :], vec[:, MU, :], vec[:, SC, :], ALU.mult
                )
                nc.vector.tensor_tensor(
                    vec[:, SH, :], gb_sb[:, L + l, :], vec[:, T0, :], ALU.subtract
                )
                # broadcast scale|shift across partitions via ones-matmul
                bc_ps = psbc.tile([P, 2 * P], F32, tag="bc")
                nc.tensor.matmul(
                    out=bc_ps[:], lhsT=ones1[:], rhs=vec[:, SC : SH + 1, :],
                    start=True, stop=True,
                )
                screp = wp.tile([P, 2, P], F32, tag="screp")
                nc.vector.tensor_copy(screp[:], bc_ps[:])

                # ---- BN apply + relu + residual (in place on agg_sb) ----
                nc.vector.tensor_tensor(
                    agg_sb[:],
                    agg_sb[:],
                    screp[:, 0:1, :].to_broadcast([P, TPC, D]),
                    ALU.mult,
                )
                nc.vector.tensor_tensor(
                    agg_sb[:],
                    agg_sb[:],
                    screp[:, 1:2, :].to_broadcast([P, TPC, D]),
                    ALU.add,
                )
                nc.scalar.activation(agg_sb[:], agg_sb[:], AF.Relu)
                nc.vector.tensor_tensor(x_sb[:], x_sb[:], agg_sb[:], ALU.add)

            nc.sync.dma_start(out_d[:].rearrange("(t p) f -> p t f", p=P), x_sb[:])

    nc.compile()
    return nc


_CACHE = {}


def _get_nc(cfg, T1s, T2s):
    key = (cfg.N, cfg.E, cfg.L, cfg.C, cfg.BPC, cfg.KG, cfg.L1, T1s, T2s)
    if key not in _CACHE:
        _CACHE[key] = _build(cfg, T1s, T2s)
    return _CACHE[key]


def run(cfg, inputs, trace=False):
    in_maps, meta = _preprocess(cfg, **inputs)
    nc = _get_nc(cfg, meta["T1s"], meta["T2s"])
    res = run_bass_kernel_spmd(nc, in_maps, core_ids=list(range(cfg.C)), trace=trace)
    newlocal = meta["newlocal"]
    xfull = np.empty((cfg.N, cfg.D), np.float32)
    for c in range(cfg.C):
        ids = np.arange(c * cfg.NSH, (c + 1) * cfg.NSH)
        xfull[ids] = res.results[c]["out"][newlocal[ids]]
    return xfull, res


def kernel(x, edge_index, W, b, gamma, beta):
    cfg = Cfg(N=50000, E=800000, D=128, L=3, C=8, bpc=7, kg=8)
    out, _ = run(
        cfg, dict(x=x, edge_index=edge_index, W=W, b=b, gamma=gamma, beta=beta)
    )
    return out


# revision 12
# speedup vs baseline: 1.1920x; 1.0410x over previous
"""GCN message-passing kernel for 8 Trainium2 NeuronCores.

Strategy (graph/data parallel, per the sharding hint):
  - Destination nodes are sharded across the 8 cores in contiguous ranges.
  - Within each core, its destinations are dealt (by in-degree, snake order)
    into 128-wide blocks so per-block edge counts are balanced across
    blocks AND cores (the SPMD program has compile-time-fixed loop bounds).
  - Per layer: each core computes hs = dinv * (x W^T + b) for its own node
    shard, downcasts to bf16. The shard is split in two halves (local rows
    [0,L1) and [L1,NPAD)); each half is AllGathered into its own Shared-HBM
    table (T1: C*L1 rows, T2: C*(NPAD-L1) rows; both < 32768 so int16
    gather indices cover them without a hi/lo base split).
  - The gpsimd instruction stream interleaves the two collectives with the
    gather calls so SWDGE descriptor generation (the bottleneck engine)
    starts as soon as T1 is ready and never waits for T2:
        AG-A, G1(c0), G1(c1), AG-B, G2(c0), MM(c0), G1(c2), G2(c1), ...
  - Messages are fetched with batched indirect DMA gathers and scatter-added
    per destination block with one-hot matmuls accumulated in PSUM:
        agg_block[d, f] += S_tile[e, d]^T @ msg_tile[e, f]
  - BN statistics (sum, sum of squares) are computed with mask-vector
    matmuls over the aggregated blocks and AllReduced across cores; the
    apply (scale/shift + relu + residual) runs on full-shard DVE/ACT ops.

kernel(**inputs) takes the FULL inputs and returns the FULL output.
"""

import numpy as np
import ml_dtypes

import concourse.bacc as bacc
import concourse.bass as bass
import concourse.mybir as mybir
import concourse.tile as tile
from concourse.bass_utils import run_bass_kernel_spmd
from concourse.masks import make_identity

P = 128
F32 = mybir.dt.float32
BF16 = mybir.dt.bfloat16
AF = mybir.ActivationFunctionType
ALU = mybir.AluOpType


class Cfg:
    def __init__(self, N, E, D, L, C, bpc, kg=8, bn_eps=1e-5, l1=3968):
        assert D == 128
        self.N, self.E, self.D, self.L, self.C = N, E, D, L, C
        self.NSH = N // C                      # real nodes per core
        assert self.NSH * C == N
        self.TPC = (self.NSH + P - 1) // P     # node tiles (blocks) per core
        self.NPAD = self.TPC * P               # padded nodes per core
        assert self.NSH < self.NPAD, "need at least one guaranteed-zero pad row"
        self.L1 = l1                           # local-row split point (A half)
        assert l1 % P == 0 and 0 < l1 < self.NPAD
        self.TA = l1 // P                      # A-half blocks
        assert C * l1 <= 32768 and C * (self.NPAD - l1) <= 32768
        self.TR1 = C * l1                      # T1 table rows
        self.TR2 = C * (self.NPAD - l1)        # T2 table rows
        self.BPC = bpc                         # blocks per chunk
        self.chunks = [
            list(range(i, min(i + bpc, self.TPC))) for i in range(0, self.TPC, bpc)
        ]
        self.BN_EPS = bn_eps
        self.KG = kg  # max idxs per dma_gather call (in 128-edge tiles)


def _preprocess(cfg, x, edge_index, W, b, gamma, beta):
    """All index/layout work on the host. Returns per-core in_maps and the
    (identical across cores) compile-time tile structure."""
    N, C, NSH, NPAD, TPC = cfg.N, cfg.C, cfg.NSH, cfg.NPAD, cfg.TPC
    L1, TA = cfg.L1, cfg.TA
    row = np.asarray(edge_index[0], dtype=np.int64)
    col = np.asarray(edge_index[1], dtype=np.int64)
    x = np.asarray(x, dtype=np.float32)
    deg = np.bincount(row, minlength=N).astype(np.float32)  # out-degree
    deg_in = np.bincount(col, minlength=N)

    # Pass 1: snake-deal destinations (sorted by in-degree desc) into TPC
    # blocks; this only FIXES each node's A/B half (block < TA -> half A,
    # i.e. the node's hs row lands in table T1). Halves then determine the
    # per-node split of in-edges by source table (w1/w2), which pass 2
    # balances per block under hard tile caps.
    last_r = (NSH - 1) // TPC
    halfb = np.empty(N, np.int64)
    for c in range(C):
        ids = np.arange(c * NSH, (c + 1) * NSH)
        order = ids[np.argsort(-deg_in[ids], kind="stable")]
        i = np.arange(NSH)
        r, j = i // TPC, i % TPC
        blk = np.where((r % 2 == 1) & (r != last_r), TPC - 1 - j, j)
        halfb[order] = (blk >= TA).astype(np.int64)
    w1 = np.bincount(col[halfb[row] == 0], minlength=N).astype(np.int64)
    w2 = np.bincount(col[halfb[row] == 1], minlength=N).astype(np.int64)

    # Pass 2: within each half, greedy 2D deal balancing (w1, w2) per
    # block. Rank 127 of blocks TA-1 and TPC-1 stays empty: guaranteed
    # all-zero gather targets for index padding in each table.
    newlocal = np.empty(N, np.int64)
    cap1, cap2 = 11 * P, 6 * P
    maskvs = []
    for c in range(C):
        ids = np.arange(c * NSH, (c + 1) * NSH)
        cnt = np.zeros(TPC, np.int64)
        l1b = np.zeros(TPC, np.int64)
        l2b = np.zeros(TPC, np.int64)
        capn = np.full(TPC, P, np.int64)
        capn[TA - 1] = P - 1
        capn[TPC - 1] = P - 1
        for h, blo, bhi in ((0, 0, TA), (1, TA, TPC)):
            sel = ids[halfb[ids] == h]
            o = np.argsort(-(w1[sel] + w2[sel]), kind="stable")
            bb = np.arange(blo, bhi)
            for v in sel[o]:
                open_ = bb[cnt[bb] < capn[bb]]
                m = np.maximum(
                    (l1b[open_] + w1[v]) * cap2, (l2b[open_] + w2[v]) * cap1
                )
                best = open_[np.argmin(m)]
                newlocal[v] = best * P + cnt[best]
                cnt[best] += 1
                l1b[best] += w1[v]
                l2b[best] += w2[v]
        maskvs.append(
            (np.arange(P)[:, None] < cnt[None, :]).astype(np.float32)
        )
    Z1 = (TA - 1) * P + P - 1           # local row, < L1, always zero
    Z2 = (TPC - 1) * P + P - 1 - L1     # T2-local row, always zero

    src_local = newlocal[row]
    src_core = row // NSH
    e_core = col // NSH
    e_blk = newlocal[col] // P
    e_rank = newlocal[col] % P
    in_t1 = src_local < L1
    idx1_full = src_core * L1 + src_local            # valid where in_t1
    idx2_full = src_core * (NPAD - L1) + (src_local - L1)

    # common tile structure: T1s/T2s tiles per block, max over cores/blocks
    per = {}
    T1s, T2s = 1, 1
    for c in range(C):
        selc = e_core == c
        for first in (True, False):
            sel = selc & (in_t1 == first)
            srcs = (idx1_full if first else idx2_full)[sel]
            blks, ranks = e_blk[sel], e_rank[sel]
            o = np.argsort(blks, kind="stable")
            srcs, blks, ranks = srcs[o], blks[o], ranks[o]
            starts = np.searchsorted(blks, np.arange(TPC))
            ends = np.searchsorted(blks, np.arange(TPC) + 1)
            per[(c, first)] = (srcs, ranks, starts, ends)
            m = int((-((ends - starts) // -P)).max())
            if first:
                T1s = max(T1s, m)
            else:
                T2s = max(T2s, m)
    TS = T1s + T2s
    NT = TPC * TS
    in_maps = []
    Wt = np.ascontiguousarray(np.transpose(np.asarray(W, np.float32), (0, 2, 1)))
    bT = np.ascontiguousarray(np.asarray(b, np.float32).T)

    def _wrap16(idx):
        w = idx.reshape(-1, 16).T.astype(np.int16)
        return np.ascontiguousarray(np.tile(w, (8, 1)))

    for c in range(C):
        idx_1 = np.full(TPC * T1s * P, Z1, np.int64)
        idx_2 = np.full(TPC * T2s * P, Z2, np.int64)
        # one-hot S matrices, block-contiguous: smat[e, (b*TS + t)*P + d]
        smat = np.zeros((P, NT * P), ml_dtypes.bfloat16)
        o1 = o2 = 0
        for ch in cfg.chunks:
            for bidx in ch:
                srcs, ranks, st, en = per[(c, True)]
                cnt = en[bidx] - st[bidx]
                idx_1[o1 : o1 + cnt] = srcs[st[bidx]:en[bidx]]
                pos = np.arange(cnt)
                rr = ranks[st[bidx]:en[bidx]]
                smat[pos % P, (bidx * TS + pos // P) * P + rr] = 1.0
                o1 += T1s * P
            for bidx in ch:
                srcs, ranks, st, en = per[(c, False)]
                cnt = en[bidx] - st[bidx]
                idx_2[o2 : o2 + cnt] = srcs[st[bidx]:en[bidx]]
                pos = np.arange(cnt)
                rr = ranks[st[bidx]:en[bidx]]
                smat[pos % P, (bidx * TS + T1s + pos // P) * P + rr] = 1.0
                o2 += T2s * P

        ids = np.arange(c * NSH, (c + 1) * NSH)
        xin = np.zeros((NPAD, cfg.D), np.float32)
        xin[newlocal[ids]] = x[ids]
        degT = np.zeros((P, TPC), np.float32)
        degT[newlocal[ids] % P, newlocal[ids] // P] = deg[ids]

        in_maps.append(
            {
                "xin": xin,
                "wt": Wt,
                "bT": bT,
                "gamma": np.asarray(gamma, np.float32),
                "beta": np.asarray(beta, np.float32),
                "degT": degT,
                "maskv": maskvs[c],
                "smat": smat,
                "idx1": _wrap16(idx_1),
                "idx2": _wrap16(idx_2),
            }
        )

    meta = dict(T1s=T1s, T2s=T2s, NT=NT, newlocal=newlocal)
    return in_maps, meta


def _build(cfg, T1s, T2s):
    """Build the SPMD Bass program (identical for all cores)."""
    N, D, L, C = cfg.N, cfg.D, cfg.L, cfg.C
    TPC, NPAD, L1, TA = cfg.TPC, cfg.NPAD, cfg.L1, cfg.TA
    TS = T1s + T2s
    NT = TPC * TS

    nc = bacc.Bacc("TRN2", target_bir_lowering=False, debug=False, num_devices=C)

    xin = nc.dram_tensor("xin", [NPAD, D], F32, kind="ExternalInput")
    wt = nc.dram_tensor("wt", [L, D, D], F32, kind="ExternalInput")
    bT = nc.dram_tensor("bT", [D, L], F32, kind="ExternalInput")
    gamma_d = nc.dram_tensor("gamma", [L, D], F32, kind="ExternalInput")
    beta_d = nc.dram_tensor("beta", [L, D], F32, kind="ExternalInput")
    degT = nc.dram_tensor("degT", [P, TPC], F32, kind="ExternalInput")
    maskv_d = nc.dram_tensor("maskv", [P, TPC], F32, kind="ExternalInput")
    smat_d = nc.dram_tensor("smat", [P, NT * P], BF16, kind="ExternalInput")
    idx1_d = nc.dram_tensor(
        "idx1", [P, TPC * T1s * P // 16], mybir.dt.int16, kind="ExternalInput"
    )
    idx2_d = nc.dram_tensor(
        "idx2", [P, TPC * T2s * P // 16], mybir.dt.int16, kind="ExternalInput"
    )
    out_d = nc.dram_tensor("out", [NPAD, D], F32, kind="ExternalOutput")

    rg = [list(range(C))]

    with tile.TileContext(nc) as tc:
        with (
            tc.tile_pool(name="persist", bufs=1) as pp,
            tc.tile_pool(name="msgp", bufs=32) as msgp,
            tc.tile_pool(name="sp", bufs=4) as sp,
            tc.tile_pool(name="work", bufs=4) as wp,
            tc.tile_pool(name="psblk", bufs=2, space="PSUM") as psblk,
            tc.tile_pool(name="psmisc", bufs=3, space="PSUM") as psmisc,
            tc.tile_pool(name="psbc", bufs=1, space="PSUM") as psbc,
            tc.tile_pool(name="psstat", bufs=2, space="PSUM") as psstat,
            tc.tile_pool(name="dram", bufs=1, space="DRAM") as dp,
        ):
            # ---- persistent loads ----
            x_sb = pp.tile([P, TPC, D], F32)
            nc.sync.dma_start(x_sb[:], xin[:].rearrange("(t p) f -> p t f", p=P))
            wt_sb = pp.tile([P, L, D], F32)
            for l in range(L):
                nc.sync.dma_start(wt_sb[:, l, :], wt[l, :, :])
            bT_sb = pp.tile([P, L], F32)
            nc.sync.dma_start(bT_sb[:], bT[:])
            gb_sb = pp.tile([1, 2 * L, D], F32)  # gamma/beta rows on partition 0
            for l in range(L):
                nc.sync.dma_start(gb_sb[:, l, :], gamma_d[l : l + 1, :])
                nc.sync.dma_start(gb_sb[:, L + l, :], beta_d[l : l + 1, :])
            deg_sb = pp.tile([P, TPC], F32)
            nc.sync.dma_start(deg_sb[:], degT[:])
            maskv_sb = pp.tile([P, TPC], F32)
            nc.sync.dma_start(maskv_sb[:], maskv_d[:])
            idx1_sb = pp.tile([P, TPC * T1s * P // 16], mybir.dt.int16)
            nc.sync.dma_start(idx1_sb[:], idx1_d[:])
            idx2_sb = pp.tile([P, TPC * T2s * P // 16], mybir.dt.int16)
            nc.sync.dma_start(idx2_sb[:], idx2_d[:])
            ident = pp.tile([P, P], F32)
            make_identity(nc, ident[:])
            ones1 = pp.tile([1, P], F32)
            nc.vector.memset(ones1[:], 1.0)

            # dinv = (deg > 0) / sqrt(max(deg, 1))
            dinv_sb = pp.tile([P, TPC], F32)
            t_a = wp.tile([P, TPC], F32, tag="dinv")
            nc.vector.tensor_scalar_max(t_a[:], deg_sb[:], 1.0)
            t_b = wp.tile([P, TPC], F32, tag="dinv")
            nc.vector.reciprocal(t_b[:], t_a[:])
            t_c = wp.tile([P, TPC], F32, tag="dinv")
            nc.scalar.sqrt(t_c[:], t_b[:])
            t_d = wp.tile([P, TPC], F32, tag="dinv")
            nc.vector.tensor_scalar(t_d[:], deg_sb[:], 0.0, None, ALU.is_gt)
            nc.vector.tensor_tensor(dinv_sb[:], t_c[:], t_d[:], ALU.mult)

            agg_sb = pp.tile([P, TPC, D], F32)
            hs_sb = pp.tile([P, TPC, D], BF16)

            # DRAM buffers: local shard halves, Shared gather tables
            shardA_dr = dp.tile([L1, D], BF16)
            shardB_dr = dp.tile([NPAD - L1, D], BF16)
            table1 = nc.dram_tensor(
                "table1_sh", [cfg.TR1, D], BF16, kind="Internal", addr_space="Shared"
            )
            table2 = nc.dram_tensor(
                "table2_sh", [cfg.TR2, D], BF16, kind="Internal", addr_space="Shared"
            )
            stats_in = dp.tile([1, 2 * D], F32)
            stats_out = dp.tile([1, 2 * D], F32)

            def _hs_tiles(l, t0, t1):
                for t in range(t0, t1):
                    xT_ps = psmisc.tile([P, P], F32, tag="ps")
                    nc.tensor.transpose(xT_ps[:], x_sb[:, t, :], ident[:])
                    xT = wp.tile([P, P], F32, tag="xT")
                    nc.vector.tensor_copy(xT[:], xT_ps[:])
                    hT_ps = psmisc.tile([P, P], F32, tag="ps")
                    nc.tensor.matmul(
                        out=hT_ps[:], lhsT=wt_sb[:, l, :], rhs=xT[:],
                        start=True, stop=True,
                    )
                    hb = wp.tile([P, P], F32, tag="hb")
                    nc.scalar.activation(
                        hb[:], hT_ps[:], AF.Identity, bias=bT_sb[:, l : l + 1]
                    )
                    h_rm_ps = psmisc.tile([P, P], F32, tag="ps")
                    nc.tensor.transpose(h_rm_ps[:], hb[:], ident[:])
                    nc.scalar.activation(
                        hs_sb[:, t, :], h_rm_ps[:], AF.Identity,
                        scale=dinv_sb[:, t : t + 1],
                    )

            for l in range(L):
                # ---- hs halves + AllGathers into Shared tables ----
                # B half (smaller) first: its AllGather + T2 gathers start
                # while the A half's hs/AllGather still run.
                _hs_tiles(l, TA, TPC)
                nc.sync.dma_start(
                    shardB_dr[:].rearrange("(t p) f -> p t f", p=P),
                    hs_sb[:, TA:TPC, :],
                )
                nc.gpsimd.collective_compute(
                    "AllGather",
                    ALU.bypass,
                    ins=[shardB_dr.opt()],
                    outs=[table2[:].opt()],
                    replica_groups=rg,
                )
                _hs_tiles(l, 0, TA)
                nc.sync.dma_start(
                    shardA_dr[:].rearrange("(t p) f -> p t f", p=P),
                    hs_sb[:, 0:TA, :],
                )

                # ---- interleaved gathers + per-chunk one-hot matmuls ----
                stA_ps = psstat.tile([1, P], F32, tag="st")
                stB_ps = psstat.tile([1, P], F32, tag="st")
                KGP = cfg.KG * P
                o1 = o2 = 0
                chunk_slots = {}
                nchunks = len(cfg.chunks)

                def _gather(k, first):
                    nonlocal o1, o2
                    nb = len(cfg.chunks[k])
                    slots = chunk_slots.setdefault(k, {})
                    if first:
                        nrow, off, base, idx_sb, tbl = (
                            nb * T1s * P, 0, o1, idx1_sb, table1,
                        )
                    else:
                        nrow, off, base, idx_sb, tbl = (
                            nb * T2s * P, nb * T1s, o2, idx2_sb, table2,
                        )
                    for g0 in range(0, nrow, KGP):
                        g1 = min(g0 + KGP, nrow)
                        mt = msgp.tile([P, cfg.KG, D], BF16, tag="msg")
                        for i in range((g1 - g0) // P):
                            slots[off + g0 // P + i] = (mt, i)
                        nc.gpsimd.dma_gather(
                            mt[:, : (g1 - g0) // P, :],
                            tbl[:],
                            idx_sb[:, (base + g0) // 16 : (base + g1) // 16],
                            g1 - g0, g1 - g0, D,
                        )
                    if first:
                        o1 += nrow
                    else:
                        o2 += nrow

                def _mms(k):
                    nb = len(cfg.chunks[k])
                    slots = chunk_slots.pop(k)
                    for j, bidx in enumerate(cfg.chunks[k]):
                        ps_b = psblk.tile([P, P], F32, tag="blk")
                        s_blk = sp.tile([P, TS, P], BF16, tag="s")
                        nc.sync.dma_start(
                            s_blk[:],
                            smat_d[:, bidx * TS * P : (bidx + 1) * TS * P],
                        )
                        for t in range(TS):
                            if t < T1s:
                                mcol = j * T1s + t
                            else:
                                mcol = nb * T1s + j * T2s + (t - T1s)
                            mt, sl = slots[mcol]
                            nc.tensor.matmul(
                                out=ps_b[:], lhsT=s_blk[:, t, :], rhs=mt[:, sl, :],
                                start=(t == 0), stop=(t == TS - 1),
                            )
                        nc.scalar.activation(
                            agg_sb[:, bidx, :], ps_b[:], AF.Identity,
                            scale=dinv_sb[:, bidx : bidx + 1],
                        )
                        nc.tensor.matmul(
                            out=stA_ps[:],
                            lhsT=maskv_sb[:, bidx : bidx + 1],
                            rhs=agg_sb[:, bidx, :],
                            start=(bidx == 0), stop=(bidx == TPC - 1),
                            skip_group_check=True,
                        )
                        aggsq = wp.tile([P, P], F32, tag="aggsq")
                        nc.scalar.square(aggsq[:], agg_sb[:, bidx, :])
                        nc.tensor.matmul(
                            out=stB_ps[:],
                            lhsT=maskv_sb[:, bidx : bidx + 1],
                            rhs=aggsq[:],
                            start=(bidx == 0), stop=(bidx == TPC - 1),
                            skip_group_check=True,
                        )

                # schedule: G2(0) G2(1) AGA G1(0) MM(0) G2(2) G1(1) MM(1) ...
                # (lag-1: G2(k) tiles live until MM(k), a <=32-buf window)
                _gather(0, False)
                for k in range(nchunks):
                    if k + 1 < nchunks:
                        _gather(k + 1, False)
                    if k == 0:
                        nc.gpsimd.collective_compute(
                            "AllGather",
                            ALU.bypass,
                            ins=[shardA_dr.opt()],
                            outs=[table1[:].opt()],
                            replica_groups=rg,
                        )
                    _gather(k, True)
                    _mms(k)

                st_sb = wp.tile([1, 2, P], F32, tag="st")
                nc.vector.tensor_copy(st_sb[:, 0, :], stA_ps[:])
                nc.vector.tensor_copy(st_sb[:, 1, :], stB_ps[:])
                nc.sync.dma_start(stats_in[:], st_sb[:])
                nc.gpsimd.collective_compute(
                    "AllReduce",
                    ALU.add,
                    ins=[stats_in.opt()],
                    outs=[stats_out.opt()],
                    replica_groups=rg,
                )
                stg = wp.tile([1, 2, P], F32, tag="st")
                nc.sync.dma_start(stg[:], stats_out[:])

                # ---- scale/shift vectors on partition 0 ----
                vec = wp.tile([1, 8, P], F32, tag="vec")
                MU, MSQ, VAR, RSTD, SC, SH, T0, T1 = range(8)
                inv_n = 1.0 / float(N)
                nc.vector.tensor_scalar_mul(vec[:, MU, :], stg[:, 0, :], inv_n)
                nc.vector.tensor_scalar_mul(vec[:, MSQ, :], stg[:, 1, :], inv_n)
                nc.vector.tensor_tensor(
                    vec[:, T0, :], vec[:, MU, :], vec[:, MU, :], ALU.mult
                )
                nc.vector.tensor_tensor(
                    vec[:, VAR, :], vec[:, MSQ, :], vec[:, T0, :], ALU.subtract
                )
                nc.vector.tensor_scalar_add(vec[:, T1, :], vec[:, VAR, :], cfg.BN_EPS)
                nc.vector.reciprocal(vec[:, T0, :], vec[:, T1, :])
                nc.scalar.sqrt(vec[:, RSTD, :], vec[:, T0, :])
                nc.vector.tensor_tensor(
                    vec[:, SC, :], gb_sb[:, l, :], vec[:, RSTD, :], ALU.mult
                )
                nc.vector.tensor_tensor(
                    vec[:, T0, :], vec[:, MU, :], vec[:, SC, :], ALU.mult
                )
                nc.vector.tensor_tensor(
                    vec[:, SH, :], gb_sb[:, L + l, :], vec[:, T0, :], ALU.subtract
                )
                # broadcast scale|shift across partitions via ones-matmul
                bc_ps = psbc.tile([P, 2 * P], F32, tag="bc")
                nc.tensor.matmul(
                    out=bc_ps[:], lhsT=ones1[:], rhs=vec[:, SC : SH + 1, :],
                    start=True, stop=True,
                )
                screp = wp.tile([P, 2, P], F32, tag="screp")
                nc.vector.tensor_copy(screp[:], bc_ps[:])

                # ---- BN apply + relu + residual (in place on agg_sb) ----
                nc.vector.tensor_tensor(
                    agg_sb[:],
                    agg_sb[:],
                    screp[:, 0:1, :].to_broadcast([P, TPC, D]),
                    ALU.mult,
                )
                nc.vector.tensor_tensor(
                    agg_sb[:],
                    agg_sb[:],
                    screp[:, 1:2, :].to_broadcast([P, TPC, D]),
                    ALU.add,
                )
                nc.scalar.activation(agg_sb[:], agg_sb[:], AF.Relu)
                nc.vector.tensor_tensor(x_sb[:], x_sb[:], agg_sb[:], ALU.add)

            nc.sync.dma_start(out_d[:].rearrange("(t p) f -> p t f", p=P), x_sb[:])

    nc.compile()
    return nc


_CACHE = {}


def _get_nc(cfg, T1s, T2s):
    key = (cfg.N, cfg.E, cfg.L, cfg.C, cfg.BPC, cfg.KG, cfg.L1, T1s, T2s)
    if key not in _CACHE:
        _CACHE[key] = _build(cfg, T1s, T2s)
    return _CACHE[key]


def run(cfg, inputs, trace=False):
    in_maps, meta = _preprocess(cfg, **inputs)
    nc = _get_nc(cfg, meta["T1s"], meta["T2s"])
    res = run_bass_kernel_spmd(nc, in_maps, core_ids=list(range(cfg.C)), trace=trace)
    newlocal = meta["newlocal"]
    xfull = np.empty((cfg.N, cfg.D), np.float32)
    for c in range(cfg.C):
        ids = np.arange(c * cfg.NSH, (c + 1) * cfg.NSH)
        xfull[ids] = res.results[c]["out"][newlocal[ids]]
    return xfull, res


def kernel(x, edge_index, W, b, gamma, beta):
    cfg = Cfg(N=50000, E=800000, D=128, L=3, C=8, bpc=7, kg=8)
    out, _ = run(
        cfg, dict(x=x, edge_index=edge_index, W=W, b=b, gamma=gamma, beta=beta)
    )
    return out


# revision 18
# speedup vs baseline: 1.1937x; 1.0014x over previous
"""GCN message-passing kernel for 8 Trainium2 NeuronCores.

Strategy (graph/data parallel, per the sharding hint):
  - Destination nodes are sharded across the 8 cores in contiguous ranges.
  - Within each core, its destinations are dealt (by in-degree, snake order)
    into 128-wide blocks so per-block edge counts are balanced across
    blocks AND cores (the SPMD program has compile-time-fixed loop bounds).
  - Per layer: each core computes hs = dinv * (x W^T + b) for its own node
    shard, downcasts to bf16. The shard is split in two halves (local rows
    [0,L1) and [L1,NPAD)); each half is AllGathered into its own Shared-HBM
    table (T1: C*L1 rows, T2: C*(NPAD-L1) rows; both < 32768 so int16
    gather indices cover them without a hi/lo base split).
  - The gpsimd instruction stream interleaves the two collectives with the
    gather calls so SWDGE descriptor generation (the bottleneck engine)
    starts as soon as T1 is ready and never waits for T2:
        AG-A, G1(c0), G1(c1), AG-B, G2(c0), MM(c0), G1(c2), G2(c1), ...
  - Messages are fetched with batched indirect DMA gathers and scatter-added
    per destination block with one-hot matmuls accumulated in PSUM:
        agg_block[d, f] += S_tile[e, d]^T @ msg_tile[e, f]
  - BN statistics (sum, sum of squares) are computed with mask-vector
    matmuls over the aggregated blocks and AllReduced across cores; the
    apply (scale/shift + relu + residual) runs on full-shard DVE/ACT ops.

kernel(**inputs) takes the FULL inputs and returns the FULL output.
"""

import numpy as np
import ml_dtypes

import concourse.bacc as bacc
import concourse.bass as bass
import concourse.mybir as mybir
import concourse.tile as tile
from concourse.bass_utils import run_bass_kernel_spmd
from concourse.masks import make_identity

P = 128
F32 = mybir.dt.float32
BF16 = mybir.dt.bfloat16
AF = mybir.ActivationFunctionType
ALU = mybir.AluOpType


class Cfg:
    def __init__(self, N, E, D, L, C, bpc, kg=8, bn_eps=1e-5, l1=3968):
        assert D == 128
        self.N, self.E, self.D, self.L, self.C = N, E, D, L, C
        self.NSH = N // C                      # real nodes per core
        assert self.NSH * C == N
        self.TPC = (self.NSH + P - 1) // P     # node tiles (blocks) per core
        self.NPAD = self.TPC * P               # padded nodes per core
        assert self.NSH < self.NPAD, "need at least one guaranteed-zero pad row"
        self.L1 = l1                           # local-row split point (A half)
        assert l1 % P == 0 and 0 < l1 < self.NPAD
        self.TA = l1 // P                      # A-half blocks
        assert C * l1 <= 32768 and C * (self.NPAD - l1) <= 32768
        self.TR1 = C * l1                      # T1 table rows
        self.TR2 = C * (self.NPAD - l1)        # T2 table rows
        self.BPC = bpc                         # blocks per chunk
        self.chunks = [
            list(range(i, min(i + bpc, self.TPC))) for i in range(0, self.TPC, bpc)
        ]
        self.BN_EPS = bn_eps
        self.KG = kg  # max idxs per dma_gather call (in 128-edge tiles)


def _preprocess(cfg, x, edge_index, W, b, gamma, beta):
    """All index/layout work on the host. Returns per-core in_maps and the
    (identical across cores) compile-time tile structure."""
    N, C, NSH, NPAD, TPC = cfg.N, cfg.C, cfg.NSH, cfg.NPAD, cfg.TPC
    L1, TA = cfg.L1, cfg.TA
    row = np.asarray(edge_index[0], dtype=np.int64)
    col = np.asarray(edge_index[1], dtype=np.int64)
    x = np.asarray(x, dtype=np.float32)
    deg = np.bincount(row, minlength=N).astype(np.float32)  # out-degree
    deg_in = np.bincount(col, minlength=N)

    # Pass 1: snake-deal destinations (sorted by in-degree desc) into TPC
    # blocks; this only FIXES each node's A/B half (block < TA -> half A,
    # i.e. the node's hs row lands in table T1). Halves then determine the
    # per-node split of in-edges by source table (w1/w2), which pass 2
    # balances per block under hard tile caps.
    last_r = (NSH - 1) // TPC
    halfb = np.empty(N, np.int64)
    for c in range(C):
        ids = np.arange(c * NSH, (c + 1) * NSH)
        order = ids[np.argsort(-deg_in[ids], kind="stable")]
        i = np.arange(NSH)
        r, j = i // TPC, i % TPC
        blk = np.where((r % 2 == 1) & (r != last_r), TPC - 1 - j, j)
        halfb[order] = (blk >= TA).astype(np.int64)
    w1 = np.bincount(col[halfb[row] == 0], minlength=N).astype(np.int64)
    w2 = np.bincount(col[halfb[row] == 1], minlength=N).astype(np.int64)

    # Pass 2: within each half, greedy 2D deal balancing (w1, w2) per
    # block. Rank 127 of blocks TA-1 and TPC-1 stays empty: guaranteed
    # all-zero gather targets for index padding in each table.
    newlocal = np.empty(N, np.int64)
    cap1, cap2 = 11 * P, 6 * P
    maskvs = []
    for c in range(C):
        ids = np.arange(c * NSH, (c + 1) * NSH)
        cnt = np.zeros(TPC, np.int64)
        l1b = np.zeros(TPC, np.int64)
        l2b = np.zeros(TPC, np.int64)
        capn = np.full(TPC, P, np.int64)
        capn[TA - 1] = P - 1
        capn[TPC - 1] = P - 1
        for h, blo, bhi in ((0, 0, TA), (1, TA, TPC)):
            sel = ids[halfb[ids] == h]
            o = np.argsort(-(w1[sel] + w2[sel]), kind="stable")
            bb = np.arange(blo, bhi)
            for v in sel[o]:
                open_ = bb[cnt[bb] < capn[bb]]
                m = np.maximum(
                    (l1b[open_] + w1[v]) * cap2, (l2b[open_] + w2[v]) * cap1
                )
                best = open_[np.argmin(m)]
                newlocal[v] = best * P + cnt[best]
                cnt[best] += 1
                l1b[best] += w1[v]
                l2b[best] += w2[v]
        maskvs.append(
            (np.arange(P)[:, None] < cnt[None, :]).astype(np.float32)
        )
    Z1 = (TA - 1) * P + P - 1           # local row, < L1, always zero
    Z2 = (TPC - 1) * P + P - 1 - L1     # T2-local row, always zero

    src_local = newlocal[row]
    src_core = row // NSH
    e_core = col // NSH
    e_blk = newlocal[col] // P
    e_rank = newlocal[col] % P
    in_t1 = src_local < L1
    idx1_full = src_core * L1 + src_local            # valid where in_t1
    idx2_full = src_core * (NPAD - L1) + (src_local - L1)

    # common tile structure: T1s/T2s tiles per block, max over cores/blocks
    per = {}
    T1s, T2s = 1, 1
    for c in range(C):
        selc = e_core == c
        for first in (True, False):
            sel = selc & (in_t1 == first)
            srcs = (idx1_full if first else idx2_full)[sel]
            blks, ranks = e_blk[sel], e_rank[sel]
            o = np.argsort(blks, kind="stable")
            srcs, blks, ranks = srcs[o], blks[o], ranks[o]
            starts = np.searchsorted(blks, np.arange(TPC))
            ends = np.searchsorted(blks, np.arange(TPC) + 1)
            per[(c, first)] = (srcs, ranks, starts, ends)
            m = int((-((ends - starts) // -P)).max())
            if first:
                T1s = max(T1s, m)
            else:
                T2s = max(T2s, m)
    TS = T1s + T2s
    NT = TPC * TS
    in_maps = []
    Wt = np.ascontiguousarray(np.transpose(np.asarray(W, np.float32), (0, 2, 1)))
    bT = np.ascontiguousarray(np.asarray(b, np.float32).T)

    def _wrap16(idx):
        w = idx.reshape(-1, 16).T.astype(np.int16)
        return np.ascontiguousarray(np.tile(w, (8, 1)))

    for c in range(C):
        idx_1 = np.full(TPC * T1s * P, Z1, np.int64)
        idx_2 = np.full(TPC * T2s * P, Z2, np.int64)
        # one-hot S matrices, block-contiguous: smat[e, (b*TS + t)*P + d]
        smat = np.zeros((P, NT * P), ml_dtypes.bfloat16)
        o1 = o2 = 0
        for ch in cfg.chunks:
            for bidx in ch:
                srcs, ranks, st, en = per[(c, True)]
                cnt = en[bidx] - st[bidx]
                idx_1[o1 : o1 + cnt] = srcs[st[bidx]:en[bidx]]
                pos = np.arange(cnt)
                rr = ranks[st[bidx]:en[bidx]]
                smat[pos % P, (bidx * TS + pos // P) * P + rr] = 1.0
                o1 += T1s * P
            for bidx in ch:
                srcs, ranks, st, en = per[(c, False)]
                cnt = en[bidx] - st[bidx]
                idx_2[o2 : o2 + cnt] = srcs[st[bidx]:en[bidx]]
                pos = np.arange(cnt)
                rr = ranks[st[bidx]:en[bidx]]
                smat[pos % P, (bidx * TS + T1s + pos // P) * P + rr] = 1.0
                o2 += T2s * P

        ids = np.arange(c * NSH, (c + 1) * NSH)
        xin = np.zeros((NPAD, cfg.D), np.float32)
        xin[newlocal[ids]] = x[ids]
        degT = np.zeros((P, TPC), np.float32)
        degT[newlocal[ids] % P, newlocal[ids] // P] = deg[ids]

        in_maps.append(
            {
                "xin": xin,
                "wt": Wt,
                "bT": bT,
                "gamma": np.asarray(gamma, np.float32),
                "beta": np.asarray(beta, np.float32),
                "degT": degT,
                "maskv": maskvs[c],
                "smat": smat,
                "idx1": _wrap16(idx_1),
                "idx2": _wrap16(idx_2),
            }
        )

    meta = dict(T1s=T1s, T2s=T2s, NT=NT, newlocal=newlocal)
    return in_maps, meta


def _build(cfg, T1s, T2s):
    """Build the SPMD Bass program (identical for all cores)."""
    N, D, L, C = cfg.N, cfg.D, cfg.L, cfg.C
    TPC, NPAD, L1, TA = cfg.TPC, cfg.NPAD, cfg.L1, cfg.TA
    TS = T1s + T2s
    NT = TPC * TS

    nc = bacc.Bacc("TRN2", target_bir_lowering=False, debug=False, num_devices=C)

    xin = nc.dram_tensor("xin", [NPAD, D], F32, kind="ExternalInput")
    wt = nc.dram_tensor("wt", [L, D, D], F32, kind="ExternalInput")
    bT = nc.dram_tensor("bT", [D, L], F32, kind="ExternalInput")
    gamma_d = nc.dram_tensor("gamma", [L, D], F32, kind="ExternalInput")
    beta_d = nc.dram_tensor("beta", [L, D], F32, kind="ExternalInput")
    degT = nc.dram_tensor("degT", [P, TPC], F32, kind="ExternalInput")
    maskv_d = nc.dram_tensor("maskv", [P, TPC], F32, kind="ExternalInput")
    smat_d = nc.dram_tensor("smat", [P, NT * P], BF16, kind="ExternalInput")
    idx1_d = nc.dram_tensor(
        "idx1", [P, TPC * T1s * P // 16], mybir.dt.int16, kind="ExternalInput"
    )
    idx2_d = nc.dram_tensor(
        "idx2", [P, TPC * T2s * P // 16], mybir.dt.int16, kind="ExternalInput"
    )
    out_d = nc.dram_tensor("out", [NPAD, D], F32, kind="ExternalOutput")

    rg = [list(range(C))]

    with tile.TileContext(nc) as tc:
        with (
            tc.tile_pool(name="persist", bufs=1) as pp,
            tc.tile_pool(name="msgp", bufs=32) as msgp,
            tc.tile_pool(name="sp", bufs=4) as sp,
            tc.tile_pool(name="work", bufs=4) as wp,
            tc.tile_pool(name="work2", bufs=2) as wp2,
            tc.tile_pool(name="psblk", bufs=2, space="PSUM") as psblk,
            tc.tile_pool(name="psmisc", bufs=3, space="PSUM") as psmisc,
            tc.tile_pool(name="psbc", bufs=1, space="PSUM") as psbc,
            tc.tile_pool(name="psstat", bufs=2, space="PSUM") as psstat,
            tc.tile_pool(name="dram", bufs=1, space="DRAM") as dp,
        ):
            # ---- persistent loads ----
            x_sb = pp.tile([P, TPC, D], F32)
            nc.sync.dma_start(x_sb[:], xin[:].rearrange("(t p) f -> p t f", p=P))
            wt_sb = pp.tile([P, L, D], F32)
            for l in range(L):
                nc.sync.dma_start(wt_sb[:, l, :], wt[l, :, :])
            bT_sb = pp.tile([P, L], F32)
            nc.sync.dma_start(bT_sb[:], bT[:])
            gb_sb = pp.tile([1, 2 * L, D], F32)  # gamma/beta rows on partition 0
            for l in range(L):
                nc.sync.dma_start(gb_sb[:, l, :], gamma_d[l : l + 1, :])
                nc.sync.dma_start(gb_sb[:, L + l, :], beta_d[l : l + 1, :])
            deg_sb = pp.tile([P, TPC], F32)
            nc.sync.dma_start(deg_sb[:], degT[:])
            maskv_sb = pp.tile([P, TPC], F32)
            nc.sync.dma_start(maskv_sb[:], maskv_d[:])
            idx1_sb = pp.tile([P, TPC * T1s * P // 16], mybir.dt.int16)
            nc.sync.dma_start(idx1_sb[:], idx1_d[:])
            idx2_sb = pp.tile([P, TPC * T2s * P // 16], mybir.dt.int16)
            nc.sync.dma_start(idx2_sb[:], idx2_d[:])
            ident = pp.tile([P, P], F32)
            make_identity(nc, ident[:])
            ones1 = pp.tile([1, P], F32)
            nc.vector.memset(ones1[:], 1.0)

            # dinv = (deg > 0) / sqrt(max(deg, 1))
            dinv_sb = pp.tile([P, TPC], F32)
            t_a = wp.tile([P, TPC], F32, tag="dinv")
            nc.vector.tensor_scalar_max(t_a[:], deg_sb[:], 1.0)
            t_b = wp.tile([P, TPC], F32, tag="dinv")
            nc.vector.reciprocal(t_b[:], t_a[:])
            t_c = wp.tile([P, TPC], F32, tag="dinv")
            nc.scalar.sqrt(t_c[:], t_b[:])
            t_d = wp.tile([P, TPC], F32, tag="dinv")
            nc.vector.tensor_scalar(t_d[:], deg_sb[:], 0.0, None, ALU.is_gt)
            nc.vector.tensor_tensor(dinv_sb[:], t_c[:], t_d[:], ALU.mult)

            agg_sb = pp.tile([P, TPC, D], F32)
            hs_sb = pp.tile([P, TPC, D], BF16)

            # DRAM buffers: local shard halves, Shared gather tables
            shardA_dr = dp.tile([L1, D], BF16)
            shardB_dr = dp.tile([NPAD - L1, D], BF16)
            table1 = nc.dram_tensor(
                "table1_sh", [cfg.TR1, D], BF16, kind="Internal", addr_space="Shared"
            )
            table2 = nc.dram_tensor(
                "table2_sh", [cfg.TR2, D], BF16, kind="Internal", addr_space="Shared"
            )
            stats_in = dp.tile([1, 2 * D], F32)
            stats_out = dp.tile([1, 2 * D], F32)

            def _hs_tiles(l, t0, t1):
                for t in range(t0, t1):
                    xT_ps = psmisc.tile([P, P], F32, tag="ps")
                    nc.tensor.transpose(xT_ps[:], x_sb[:, t, :], ident[:])
                    xT = wp.tile([P, P], F32, tag="xT")
                    nc.vector.tensor_copy(xT[:], xT_ps[:])
                    hT_ps = psmisc.tile([P, P], F32, tag="ps")
                    nc.tensor.matmul(
                        out=hT_ps[:], lhsT=wt_sb[:, l, :], rhs=xT[:],
                        start=True, stop=True,
                    )
                    hb = wp.tile([P, P], F32, tag="hb")
                    nc.scalar.activation(
                        hb[:], hT_ps[:], AF.Identity, bias=bT_sb[:, l : l + 1]
                    )
                    h_rm_ps = psmisc.tile([P, P], F32, tag="ps")
                    nc.tensor.transpose(h_rm_ps[:], hb[:], ident[:])
                    nc.scalar.activation(
                        hs_sb[:, t, :], h_rm_ps[:], AF.Identity,
                        scale=dinv_sb[:, t : t + 1],
                    )

            for l in range(L):
                # ---- hs halves + AllGathers into Shared tables ----
                # B half (smaller) first: its AllGather + T2 gathers start
                # while the A half's hs/AllGather still run.
                _hs_tiles(l, TA, TPC)
                nc.sync.dma_start(
                    shardB_dr[:].rearrange("(t p) f -> p t f", p=P),
                    hs_sb[:, TA:TPC, :],
                )
                nc.gpsimd.collective_compute(
                    "AllGather",
                    ALU.bypass,
                    ins=[shardB_dr.opt()],
                    outs=[table2[:].opt()],
                    replica_groups=rg,
                )
                _hs_tiles(l, 0, TA)
                nc.sync.dma_start(
                    shardA_dr[:].rearrange("(t p) f -> p t f", p=P),
                    hs_sb[:, 0:TA, :],
                )

                # ---- interleaved gathers + per-chunk one-hot matmuls ----
                stA_ps = psstat.tile([1, P], F32, tag="st")
                stB_ps = psstat.tile([1, P], F32, tag="st")
                KGP = cfg.KG * P
                o1 = o2 = 0
                chunk_slots = {}
                nchunks = len(cfg.chunks)

                def _gather(k, first):
                    nonlocal o1, o2
                    nb = len(cfg.chunks[k])
                    slots = chunk_slots.setdefault(k, {})
                    if first:
                        nrow, off, base, idx_sb, tbl = (
                            nb * T1s * P, 0, o1, idx1_sb, table1,
                        )
                    else:
                        nrow, off, base, idx_sb, tbl = (
                            nb * T2s * P, nb * T1s, o2, idx2_sb, table2,
                        )
                    for g0 in range(0, nrow, KGP):
                        g1 = min(g0 + KGP, nrow)
                        mt = msgp.tile([P, cfg.KG, D], BF16, tag="msg")
                        for i in range((g1 - g0) // P):
                            slots[off + g0 // P + i] = (mt, i)
                        nc.gpsimd.dma_gather(
                            mt[:, : (g1 - g0) // P, :],
                            tbl[:],
                            idx_sb[:, (base + g0) // 16 : (base + g1) // 16],
                            g1 - g0, g1 - g0, D,
                        )
                    if first:
                        o1 += nrow
                    else:
                        o2 += nrow

                def _mms(k):
                    nb = len(cfg.chunks[k])
                    slots = chunk_slots.pop(k)
                    for j, bidx in enumerate(cfg.chunks[k]):
                        ps_b = psblk.tile([P, P], F32, tag="blk")
                        s_blk = sp.tile([P, TS, P], BF16, tag="s")
                        nc.sync.dma_start(
                            s_blk[:],
                            smat_d[:, bidx * TS * P : (bidx + 1) * TS * P],
                        )
                        for t in range(TS):
                            if t < T1s:
                                mcol = j * T1s + t
                            else:
                                mcol = nb * T1s + j * T2s + (t - T1s)
                            mt, sl = slots[mcol]
                            nc.tensor.matmul(
                                out=ps_b[:], lhsT=s_blk[:, t, :], rhs=mt[:, sl, :],
                                start=(t == 0), stop=(t == TS - 1),
                            )
                        nc.scalar.activation(
                            agg_sb[:, bidx, :], ps_b[:], AF.Identity,
                            scale=dinv_sb[:, bidx : bidx + 1],
                        )
                        nc.tensor.matmul(
                            out=stA_ps[:],
                            lhsT=maskv_sb[:, bidx : bidx + 1],
                            rhs=agg_sb[:, bidx, :],
                            start=(bidx == 0), stop=(bidx == TPC - 1),
                            skip_group_check=True,
                        )
                        aggsq = wp.tile([P, P], F32, tag="aggsq")
                        nc.scalar.square(aggsq[:], agg_sb[:, bidx, :])
                        nc.tensor.matmul(
                            out=stB_ps[:],
                            lhsT=maskv_sb[:, bidx : bidx + 1],
                            rhs=aggsq[:],
                            start=(bidx == 0), stop=(bidx == TPC - 1),
                            skip_group_check=True,
                        )

                # schedule: G2(0) G2(1) AGA G1(0) MM(0) G2(2) G1(1) MM(1) ...
                # (lag-1: G2(k) tiles live until MM(k), a <=32-buf window)
                _gather(0, False)
                for k in range(nchunks):
                    if k + 1 < nchunks:
                        _gather(k + 1, False)
                    if k == 0:
                        nc.gpsimd.collective_compute(
                            "AllGather",
                            ALU.bypass,
                            ins=[shardA_dr.opt()],
                            outs=[table1[:].opt()],
                            replica_groups=rg,
                        )
                    _gather(k, True)
                    _mms(k)

                st_sb = wp2.tile([1, 2, P], F32, tag="st")
                nc.vector.tensor_copy(st_sb[:, 0, :], stA_ps[:])
                nc.vector.tensor_copy(st_sb[:, 1, :], stB_ps[:])
                nc.sync.dma_start(stats_in[:], st_sb[:])
                nc.gpsimd.collective_compute(
                    "AllReduce",
                    ALU.add,
                    ins=[stats_in.opt()],
                    outs=[stats_out.opt()],
                    replica_groups=rg,
                )
                stg = wp2.tile([1, 2, P], F32, tag="st")
                nc.sync.dma_start(stg[:], stats_out[:])

                # ---- scale/shift vectors on partition 0 ----
                vec = wp2.tile([1, 8, P], F32, tag="vec")
                MU, MSQ, VAR, RSTD, SC, SH, T0, T1 = range(8)
                inv_n = 1.0 / float(N)
                nc.vector.tensor_scalar_mul(vec[:, MU, :], stg[:, 0, :], inv_n)
                nc.vector.tensor_scalar_mul(vec[:, MSQ, :], stg[:, 1, :], inv_n)
                nc.vector.tensor_tensor(
                    vec[:, T0, :], vec[:, MU, :], vec[:, MU, :], ALU.mult
                )
                nc.vector.tensor_tensor(
                    vec[:, VAR, :], vec[:, MSQ, :], vec[:, T0, :], ALU.subtract
                )
                nc.vector.tensor_scalar_add(vec[:, T1, :], vec[:, VAR, :], cfg.BN_EPS)
                nc.vector.reciprocal(vec[:, T0, :], vec[:, T1, :])
                nc.scalar.sqrt(vec[:, RSTD, :], vec[:, T0, :])
                nc.vector.tensor_tensor(
                    vec[:, SC, :], gb_sb[:, l, :], vec[:, RSTD, :], ALU.mult
                )
                nc.vector.tensor_tensor(
                    vec[:, T0, :], vec[:, MU, :], vec[:, SC, :], ALU.mult
                )
                nc.vector.tensor_tensor(
                    vec[:, SH, :], gb_sb[:, L + l, :], vec[:, T0, :], ALU.subtract
                )
                # broadcast scale|shift across partitions via ones-matmul
                bc_ps = psbc.tile([P, 2 * P], F32, tag="bc")
                nc.tensor.matmul(
                    out=bc_ps[:], lhsT=ones1[:], rhs=vec[:, SC : SH + 1, :],
                    start=True, stop=True,
                )
                screp = wp2.tile([P, 2, P], F32, tag="screp")
                nc.vector.tensor_copy(screp[:], bc_ps[:])

                # ---- BN apply + relu + residual (in place on agg_sb) ----
                nc.vector.tensor_tensor(
                    agg_sb[:],
                    agg_sb[:],
                    screp[:, 0:1, :].to_broadcast([P, TPC, D]),
                    ALU.mult,
                )
                nc.vector.tensor_tensor(
                    agg_sb[:],
                    agg_sb[:],
                    screp[:, 1:2, :].to_broadcast([P, TPC, D]),
                    ALU.add,
                )
                nc.scalar.activation(agg_sb[:], agg_sb[:], AF.Relu)
                nc.vector.tensor_tensor(x_sb[:], x_sb[:], agg_sb[:], ALU.add)

            nc.sync.dma_start(out_d[:].rearrange("(t p) f -> p t f", p=P), x_sb[:])

    nc.compile()
    return nc


_CACHE = {}


def _get_nc(cfg, T1s, T2s):
    key = (cfg.N, cfg.E, cfg.L, cfg.C, cfg.BPC, cfg.KG, cfg.L1, T1s, T2s)
    if key not in _CACHE:
        _CACHE[key] = _build(cfg, T1s, T2s)
    return _CACHE[key]


def run(cfg, inputs, trace=False):
    in_maps, meta = _preprocess(cfg, **inputs)
    nc = _get_nc(cfg, meta["T1s"], meta["T2s"])
    res = run_bass_kernel_spmd(nc, in_maps, core_ids=list(range(cfg.C)), trace=trace)
    newlocal = meta["newlocal"]
    xfull = np.empty((cfg.N, cfg.D), np.float32)
    for c in range(cfg.C):
        ids = np.arange(c * cfg.NSH, (c + 1) * cfg.NSH)
        xfull[ids] = res.results[c]["out"][newlocal[ids]]
    return xfull, res


def kernel(x, edge_index, W, b, gamma, beta):
    cfg = Cfg(N=50000, E=800000, D=128, L=3, C=8, bpc=7, kg=8)
    out, _ = run(
        cfg, dict(x=x, edge_index=edge_index, W=W, b=b, gamma=gamma, beta=beta)
    )
    return out


# revision 25
# speedup vs baseline: 1.1950x; 1.0011x over previous
"""GCN message-passing kernel for 8 Trainium2 NeuronCores.

Strategy (graph/data parallel, per the sharding hint):
  - Destination nodes are sharded across the 8 cores in contiguous ranges.
  - Within each core, its destinations are dealt (by in-degree, snake order)
    into 128-wide blocks so per-block edge counts are balanced across
    blocks AND cores (the SPMD program has compile-time-fixed loop bounds).
  - Per layer: each core computes hs = dinv * (x W^T + b) for its own node
    shard, downcasts to bf16. The shard is split in two halves (local rows
    [0,L1) and [L1,NPAD)); each half is AllGathered into its own Shared-HBM
    table (T1: C*L1 rows, T2: C*(NPAD-L1) rows; both < 32768 so int16
    gather indices cover them without a hi/lo base split).
  - The gpsimd instruction stream interleaves the two collectives with the
    gather calls so SWDGE descriptor generation (the bottleneck engine)
    starts as soon as T1 is ready and never waits for T2:
        AG-A, G1(c0), G1(c1), AG-B, G2(c0), MM(c0), G1(c2), G2(c1), ...
  - Messages are fetched with batched indirect DMA gathers and scatter-added
    per destination block with one-hot matmuls accumulated in PSUM:
        agg_block[d, f] += S_tile[e, d]^T @ msg_tile[e, f]
  - BN statistics (sum, sum of squares) are computed with mask-vector
    matmuls over the aggregated blocks and AllReduced across cores; the
    apply (scale/shift + relu + residual) runs on full-shard DVE/ACT ops.

kernel(**inputs) takes the FULL inputs and returns the FULL output.
"""

import numpy as np
import ml_dtypes

import concourse.bacc as bacc
import concourse.bass as bass
import concourse.mybir as mybir
import concourse.tile as tile
from concourse.bass_utils import run_bass_kernel_spmd
from concourse.masks import make_identity

P = 128
F32 = mybir.dt.float32
BF16 = mybir.dt.bfloat16
AF = mybir.ActivationFunctionType
ALU = mybir.AluOpType


class Cfg:
    def __init__(self, N, E, D, L, C, bpc, kg=8, bn_eps=1e-5, l1=3968):
        assert D == 128
        self.N, self.E, self.D, self.L, self.C = N, E, D, L, C
        self.NSH = N // C                      # real nodes per core
        assert self.NSH * C == N
        self.TPC = (self.NSH + P - 1) // P     # node tiles (blocks) per core
        self.NPAD = self.TPC * P               # padded nodes per core
        assert self.NSH < self.NPAD, "need at least one guaranteed-zero pad row"
        self.L1 = l1                           # local-row split point (A half)
        assert l1 % P == 0 and 0 < l1 < self.NPAD
        self.TA = l1 // P                      # A-half blocks
        assert C * l1 <= 32768 and C * (self.NPAD - l1) <= 32768
        self.TR1 = C * l1                      # T1 table rows
        self.TR2 = C * (self.NPAD - l1)        # T2 table rows
        self.BPC = bpc                         # blocks per chunk
        self.chunks = [
            list(range(i, min(i + bpc, self.TPC))) for i in range(0, self.TPC, bpc)
        ]
        self.BN_EPS = bn_eps
        self.KG = kg  # max idxs per dma_gather call (in 128-edge tiles)


def _preprocess(cfg, x, edge_index, W, b, gamma, beta):
    """All index/layout work on the host. Returns per-core in_maps and the
    (identical across cores) compile-time tile structure."""
    N, C, NSH, NPAD, TPC = cfg.N, cfg.C, cfg.NSH, cfg.NPAD, cfg.TPC
    L1, TA = cfg.L1, cfg.TA
    row = np.asarray(edge_index[0], dtype=np.int64)
    col = np.asarray(edge_index[1], dtype=np.int64)
    x = np.asarray(x, dtype=np.float32)
    deg = np.bincount(row, minlength=N).astype(np.float32)  # out-degree
    deg_in = np.bincount(col, minlength=N)

    # Pass 1: snake-deal destinations (sorted by in-degree desc) into TPC
    # blocks; this only FIXES each node's A/B half (block < TA -> half A,
    # i.e. the node's hs row lands in table T1). Halves then determine the
    # per-node split of in-edges by source table (w1/w2), which pass 2
    # balances per block under hard tile caps.
    last_r = (NSH - 1) // TPC
    halfb = np.empty(N, np.int64)
    for c in range(C):
        ids = np.arange(c * NSH, (c + 1) * NSH)
        order = ids[np.argsort(-deg_in[ids], kind="stable")]
        i = np.arange(NSH)
        r, j = i // TPC, i % TPC
        blk = np.where((r % 2 == 1) & (r != last_r), TPC - 1 - j, j)
        halfb[order] = (blk >= TA).astype(np.int64)
    w1 = np.bincount(col[halfb[row] == 0], minlength=N).astype(np.int64)
    w2 = np.bincount(col[halfb[row] == 1], minlength=N).astype(np.int64)

    # Pass 2: within each half, greedy 2D deal balancing (w1, w2) per
    # block. Rank 127 of blocks TA-1 and TPC-1 stays empty: guaranteed
    # all-zero gather targets for index padding in each table.
    newlocal = np.empty(N, np.int64)
    cap1, cap2 = 11 * P, 6 * P
    maskvs = []
    for c in range(C):
        ids = np.arange(c * NSH, (c + 1) * NSH)
        cnt = np.zeros(TPC, np.int64)
        l1b = np.zeros(TPC, np.int64)
        l2b = np.zeros(TPC, np.int64)
        capn = np.full(TPC, P, np.int64)
        capn[TA - 1] = P - 1
        capn[TPC - 1] = P - 1
        for h, blo, bhi in ((0, 0, TA), (1, TA, TPC)):
            sel = ids[halfb[ids] == h]
            o = np.argsort(-(w1[sel] + w2[sel]), kind="stable")
            bb = np.arange(blo, bhi)
            for v in sel[o]:
                open_ = bb[cnt[bb] < capn[bb]]
                m = np.maximum(
                    (l1b[open_] + w1[v]) * cap2, (l2b[open_] + w2[v]) * cap1
                )
                best = open_[np.argmin(m)]
                newlocal[v] = best * P + cnt[best]
                cnt[best] += 1
                l1b[best] += w1[v]
                l2b[best] += w2[v]
        maskvs.append(
            (np.arange(P)[:, None] < cnt[None, :]).astype(np.float32)
        )
    Z1 = (TA - 1) * P + P - 1           # local row, < L1, always zero
    Z2 = (TPC - 1) * P + P - 1 - L1     # T2-local row, always zero

    src_local = newlocal[row]
    src_core = row // NSH
    e_core = col // NSH
    e_blk = newlocal[col] // P
    e_rank = newlocal[col] % P
    in_t1 = src_local < L1
    idx1_full = src_core * L1 + src_local            # valid where in_t1
    idx2_full = src_core * (NPAD - L1) + (src_local - L1)

    # common tile structure: T1s/T2s tiles per block, max over cores/blocks
    per = {}
    T1s, T2s = 1, 1
    for c in range(C):
        selc = e_core == c
        for first in (True, False):
            sel = selc & (in_t1 == first)
            srcs = (idx1_full if first else idx2_full)[sel]
            blks, ranks = e_blk[sel], e_rank[sel]
            o = np.argsort(blks, kind="stable")
            srcs, blks, ranks = srcs[o], blks[o], ranks[o]
            starts = np.searchsorted(blks, np.arange(TPC))
            ends = np.searchsorted(blks, np.arange(TPC) + 1)
            per[(c, first)] = (srcs, ranks, starts, ends)
            m = int((-((ends - starts) // -P)).max())
            if first:
                T1s = max(T1s, m)
            else:
                T2s = max(T2s, m)
    TS = T1s + T2s
    NT = TPC * TS
    in_maps = []
    Wt = np.ascontiguousarray(np.transpose(np.asarray(W, np.float32), (0, 2, 1)))
    bT = np.ascontiguousarray(np.asarray(b, np.float32).T)

    def _wrap16(idx):
        w = idx.reshape(-1, 16).T.astype(np.int16)
        return np.ascontiguousarray(np.tile(w, (8, 1)))

    for c in range(C):
        idx_1 = np.full(TPC * T1s * P, Z1, np.int64)
        idx_2 = np.full(TPC * T2s * P, Z2, np.int64)
        # one-hot S matrices, block-contiguous: smat[e, (b*TS + t)*P + d]
        smat = np.zeros((P, NT * P), ml_dtypes.bfloat16)
        o1 = o2 = 0
        for ch in cfg.chunks:
            for bidx in ch:
                srcs, ranks, st, en = per[(c, True)]
                cnt = en[bidx] - st[bidx]
                idx_1[o1 : o1 + cnt] = srcs[st[bidx]:en[bidx]]
                pos = np.arange(cnt)
                rr = ranks[st[bidx]:en[bidx]]
                smat[pos % P, (bidx * TS + pos // P) * P + rr] = 1.0
                o1 += T1s * P
            for bidx in ch:
                srcs, ranks, st, en = per[(c, False)]
                cnt = en[bidx] - st[bidx]
                idx_2[o2 : o2 + cnt] = srcs[st[bidx]:en[bidx]]
                pos = np.arange(cnt)
                rr = ranks[st[bidx]:en[bidx]]
                smat[pos % P, (bidx * TS + T1s + pos // P) * P + rr] = 1.0
                o2 += T2s * P

        ids = np.arange(c * NSH, (c + 1) * NSH)
        xin = np.zeros((NPAD, cfg.D), np.float32)
        xin[newlocal[ids]] = x[ids]
        degT = np.zeros((P, TPC), np.float32)
        degT[newlocal[ids] % P, newlocal[ids] // P] = deg[ids]

        in_maps.append(
            {
                "xin": xin,
                "wt": Wt,
                "bT": bT,
                "gamma": np.asarray(gamma, np.float32),
                "beta": np.asarray(beta, np.float32),
                "degT": degT,
                "maskv": maskvs[c],
                "smat": smat,
                "idx1": _wrap16(idx_1),
                "idx2": _wrap16(idx_2),
            }
        )

    meta = dict(T1s=T1s, T2s=T2s, NT=NT, newlocal=newlocal)
    return in_maps, meta


def _build(cfg, T1s, T2s):
    """Build the SPMD Bass program (identical for all cores)."""
    N, D, L, C = cfg.N, cfg.D, cfg.L, cfg.C
    TPC, NPAD, L1, TA = cfg.TPC, cfg.NPAD, cfg.L1, cfg.TA
    TS = T1s + T2s
    NT = TPC * TS

    nc = bacc.Bacc(
        "TRN2",
        target_bir_lowering=False,
        debug=False,
        num_devices=C,
    )

    xin = nc.dram_tensor("xin", [NPAD, D], F32, kind="ExternalInput")
    wt = nc.dram_tensor("wt", [L, D, D], F32, kind="ExternalInput")
    bT = nc.dram_tensor("bT", [D, L], F32, kind="ExternalInput")
    gamma_d = nc.dram_tensor("gamma", [L, D], F32, kind="ExternalInput")
    beta_d = nc.dram_tensor("beta", [L, D], F32, kind="ExternalInput")
    degT = nc.dram_tensor("degT", [P, TPC], F32, kind="ExternalInput")
    maskv_d = nc.dram_tensor("maskv", [P, TPC], F32, kind="ExternalInput")
    smat_d = nc.dram_tensor("smat", [P, NT * P], BF16, kind="ExternalInput")
    idx1_d = nc.dram_tensor(
        "idx1", [P, TPC * T1s * P // 16], mybir.dt.int16, kind="ExternalInput"
    )
    idx2_d = nc.dram_tensor(
        "idx2", [P, TPC * T2s * P // 16], mybir.dt.int16, kind="ExternalInput"
    )
    out_d = nc.dram_tensor("out", [NPAD, D], F32, kind="ExternalOutput")

    rg = [list(range(C))]

    with tile.TileContext(nc) as tc:
        with (
            tc.tile_pool(name="persist", bufs=1) as pp,
            tc.tile_pool(name="msgp", bufs=32) as msgp,
            tc.tile_pool(name="sp", bufs=4) as sp,
            tc.tile_pool(name="work", bufs=4) as wp,
            tc.tile_pool(name="work2", bufs=2) as wp2,
            tc.tile_pool(name="psblk", bufs=2, space="PSUM") as psblk,
            tc.tile_pool(name="psmisc", bufs=3, space="PSUM") as psmisc,
            tc.tile_pool(name="psbc", bufs=1, space="PSUM") as psbc,
            tc.tile_pool(name="psstat", bufs=2, space="PSUM") as psstat,
            tc.tile_pool(name="dram", bufs=1, space="DRAM") as dp,
        ):
            # ---- persistent loads ----
            x_sb = pp.tile([P, TPC, D], F32)
            nc.sync.dma_start(x_sb[:], xin[:].rearrange("(t p) f -> p t f", p=P))
            wt_sb = pp.tile([P, L, D], F32)
            for l in range(L):
                nc.sync.dma_start(wt_sb[:, l, :], wt[l, :, :])
            bT_sb = pp.tile([P, L], F32)
            nc.sync.dma_start(bT_sb[:], bT[:])
            gb_sb = pp.tile([1, 2 * L, D], F32)  # gamma/beta rows on partition 0
            for l in range(L):
                nc.sync.dma_start(gb_sb[:, l, :], gamma_d[l : l + 1, :])
                nc.sync.dma_start(gb_sb[:, L + l, :], beta_d[l : l + 1, :])
            deg_sb = pp.tile([P, TPC], F32)
            nc.sync.dma_start(deg_sb[:], degT[:])
            maskv_sb = pp.tile([P, TPC], F32)
            nc.sync.dma_start(maskv_sb[:], maskv_d[:])
            idx1_sb = pp.tile([P, TPC * T1s * P // 16], mybir.dt.int16)
            nc.sync.dma_start(idx1_sb[:], idx1_d[:])
            idx2_sb = pp.tile([P, TPC * T2s * P // 16], mybir.dt.int16)
            nc.sync.dma_start(idx2_sb[:], idx2_d[:])
            ident = pp.tile([P, P], F32)
            make_identity(nc, ident[:])
            ones1 = pp.tile([1, P], F32)
            nc.vector.memset(ones1[:], 1.0)

            # dinv = (deg > 0) / sqrt(max(deg, 1))
            dinv_sb = pp.tile([P, TPC], F32)
            t_a = wp.tile([P, TPC], F32, tag="dinv")
            nc.vector.tensor_scalar_max(t_a[:], deg_sb[:], 1.0)
            t_b = wp.tile([P, TPC], F32, tag="dinv")
            nc.vector.reciprocal(t_b[:], t_a[:])
            t_c = wp.tile([P, TPC], F32, tag="dinv")
            nc.scalar.sqrt(t_c[:], t_b[:])
            t_d = wp.tile([P, TPC], F32, tag="dinv")
            nc.vector.tensor_scalar(t_d[:], deg_sb[:], 0.0, None, ALU.is_gt)
            nc.vector.tensor_tensor(dinv_sb[:], t_c[:], t_d[:], ALU.mult)

            agg_sb = pp.tile([P, TPC, D], F32)
            hs_sb = pp.tile([P, TPC, D], BF16)

            # DRAM buffers: local shard halves, Shared gather tables
            shardA_dr = dp.tile([L1, D], BF16)
            shardB_dr = dp.tile([NPAD - L1, D], BF16)
            table1 = nc.dram_tensor(
                "table1_sh", [cfg.TR1, D], BF16, kind="Internal", addr_space="Shared"
            )
            table2 = nc.dram_tensor(
                "table2_sh", [cfg.TR2, D], BF16, kind="Internal", addr_space="Shared"
            )
            stats_in = dp.tile([1, 2 * D], F32)
            stats_out = dp.tile([1, 2 * D], F32)

            def _hs_tiles(l, t0, t1):
                for t in range(t0, t1):
                    xT_ps = psmisc.tile([P, P], F32, tag="ps")
                    nc.tensor.transpose(xT_ps[:], x_sb[:, t, :], ident[:])
                    xT = wp.tile([P, P], F32, tag="xT")
                    nc.vector.tensor_copy(xT[:], xT_ps[:])
                    hT_ps = psmisc.tile([P, P], F32, tag="ps")
                    nc.tensor.matmul(
                        out=hT_ps[:], lhsT=wt_sb[:, l, :], rhs=xT[:],
                        start=True, stop=True,
                    )
                    hb = wp.tile([P, P], F32, tag="hb")
                    nc.scalar.activation(
                        hb[:], hT_ps[:], AF.Identity, bias=bT_sb[:, l : l + 1]
                    )
                    h_rm_ps = psmisc.tile([P, P], F32, tag="ps")
                    nc.tensor.transpose(h_rm_ps[:], hb[:], ident[:])
                    nc.scalar.activation(
                        hs_sb[:, t, :], h_rm_ps[:], AF.Identity,
                        scale=dinv_sb[:, t : t + 1],
                    )

            for l in range(L):
                pass

                # ---- interleaved gathers + per-chunk one-hot matmuls ----
                stA_ps = psstat.tile([1, P], F32, tag="st")
                stB_ps = psstat.tile([1, P], F32, tag="st")
                KGP = cfg.KG * P
                chunk_slots = {}
                nchunks = len(cfg.chunks)

                def _gather(k, first):
                    nb = len(cfg.chunks[k])
                    slots = chunk_slots.setdefault((k, first), {})
                    ts_ = T1s if first else T2s
                    nrow = nb * ts_ * P
                    base = sum(len(c_) * ts_ * P for c_ in cfg.chunks[:k])
                    idx_sb = idx1_sb if first else idx2_sb
                    tbl = table1 if first else table2
                    for g0 in range(0, nrow, KGP):
                        g1 = min(g0 + KGP, nrow)
                        mt = msgp.tile([P, cfg.KG, D], BF16, tag="msg")
                        for i in range((g1 - g0) // P):
                            slots[g0 // P + i] = (mt, i)
                        nc.gpsimd.dma_gather(
                            mt[:, : (g1 - g0) // P, :],
                            tbl[:],
                            idx_sb[:, (base + g0) // 16 : (base + g1) // 16],
                            g1 - g0, g1 - g0, D,
                        )

                def _mms2(k):
                    # T2 partial sums -> agg_sb (unscaled); frees G2 tiles
                    slots = chunk_slots.pop((k, False))
                    for j, bidx in enumerate(cfg.chunks[k]):
                        ps2 = psmisc.tile([P, P], F32, tag="ps")
                        s2 = sp.tile([P, T2s, P], BF16, tag="s2")
                        nc.sync.dma_start(
                            s2[:],
                            smat_d[
                                :,
                                (bidx * TS + T1s) * P : (bidx + 1) * TS * P,
                            ],
                        )
                        for t in range(T2s):
                            mt, sl = slots[j * T2s + t]
                            nc.tensor.matmul(
                                out=ps2[:], lhsT=s2[:, t, :], rhs=mt[:, sl, :],
                                start=(t == 0), stop=(t == T2s - 1),
                            )
                        nc.scalar.activation(
                            agg_sb[:, bidx, :], ps2[:], AF.Identity
                        )

                def _mms1(k):
                    # T1 sums + combine with the T2 partial + dinv scale
                    slots = chunk_slots.pop((k, True))
                    for j, bidx in enumerate(cfg.chunks[k]):
                        ps_b = psblk.tile([P, P], F32, tag="blk")
                        s_blk = sp.tile([P, T1s, P], BF16, tag="s")
                        nc.sync.dma_start(
                            s_blk[:],
                            smat_d[:, bidx * TS * P : (bidx * TS + T1s) * P],
                        )
                        for t in range(T1s):
                            mt, sl = slots[j * T1s + t]
                            nc.tensor.matmul(
                                out=ps_b[:], lhsT=s_blk[:, t, :], rhs=mt[:, sl, :],
                                start=(t == 0), stop=(t == T1s - 1),
                            )
                        nc.vector.tensor_tensor(
                            agg_sb[:, bidx, :], agg_sb[:, bidx, :], ps_b[:],
                            ALU.add,
                        )
                        nc.scalar.activation(
                            agg_sb[:, bidx, :], agg_sb[:, bidx, :], AF.Identity,
                            scale=dinv_sb[:, bidx : bidx + 1],
                        )
                        nc.tensor.matmul(
                            out=stA_ps[:],
                            lhsT=maskv_sb[:, bidx : bidx + 1],
                            rhs=agg_sb[:, bidx, :],
                            start=(bidx == 0), stop=(bidx == TPC - 1),
                            skip_group_check=True,
                        )
                        aggsq = wp.tile([P, P], F32, tag="aggsq")
                        nc.scalar.square(aggsq[:], agg_sb[:, bidx, :])
                        nc.tensor.matmul(
                            out=stB_ps[:],
                            lhsT=maskv_sb[:, bidx : bidx + 1],
                            rhs=aggsq[:],
                            start=(bidx == 0), stop=(bidx == TPC - 1),
                            skip_group_check=True,
                        )

                # ---- hs halves + AllGathers into Shared tables ----
                # The first 4 G2 calls are prepare_only on SWDGE queues
                # 0..3: desc-gen runs during the BN/hs idle; the DMAs fire
                # via trigger_dma right after AG-B lands. B half (smaller)
                # goes first: its AllGather + T2 gathers overlap the A
                # half's hs/AllGather.
                _hs_tiles(l, TA, TPC)
                nc.sync.dma_start(
                    shardB_dr[:].rearrange("(t p) f -> p t f", p=P),
                    hs_sb[:, TA:TPC, :],
                )
                nc.gpsimd.collective_compute(
                    "AllGather",
                    ALU.bypass,
                    ins=[shardB_dr.opt()],
                    outs=[table2[:].opt()],
                    replica_groups=rg,
                )
                _hs_tiles(l, 0, TA)
                nc.sync.dma_start(
                    shardA_dr[:].rearrange("(t p) f -> p t f", p=P),
                    hs_sb[:, 0:TA, :],
                )

                # schedule: all T2 gathers + T2-partial matmuls first
                # (AG-A slotted mid-stream so both its wait-for-hs-A and
                # its drain hide behind G2 desc-gen), then T1 gathers +
                # combine. Each half is lag-1: tiles live one chunk.
                _gather(0, False)
                for k in range(nchunks):
                    if k + 1 < nchunks:
                        _gather(k + 1, False)
                    if k == 2:
                        nc.gpsimd.collective_compute(
                            "AllGather",
                            ALU.bypass,
                            ins=[shardA_dr.opt()],
                            outs=[table1[:].opt()],
                            replica_groups=rg,
                        )
                    _mms2(k)
                _gather(0, True)
                for k in range(nchunks):
                    if k + 1 < nchunks:
                        _gather(k + 1, True)
                    _mms1(k)

                st_sb = wp2.tile([1, 2, P], F32, tag="st")
                nc.vector.tensor_copy(st_sb[:, 0, :], stA_ps[:])
                nc.vector.tensor_copy(st_sb[:, 1, :], stB_ps[:])
                nc.sync.dma_start(stats_in[:], st_sb[:])
                nc.gpsimd.collective_compute(
                    "AllReduce",
                    ALU.add,
                    ins=[stats_in.opt()],
                    outs=[stats_out.opt()],
                    replica_groups=rg,
                )
                stg = wp2.tile([1, 2, P], F32, tag="st")
                nc.sync.dma_start(stg[:], stats_out[:])

                # ---- scale/shift vectors on partition 0 ----
                vec = wp2.tile([1, 8, P], F32, tag="vec")
                MU, MSQ, VAR, RSTD, SC, SH, T0, T1 = range(8)
                inv_n = 1.0 / float(N)
                nc.vector.tensor_scalar_mul(vec[:, MU, :], stg[:, 0, :], inv_n)
                nc.vector.tensor_scalar_mul(vec[:, MSQ, :], stg[:, 1, :], inv_n)
                nc.vector.tensor_tensor(
                    vec[:, T0, :], vec[:, MU, :], vec[:, MU, :], ALU.mult
                )
                nc.vector.tensor_tensor(
                    vec[:, VAR, :], vec[:, MSQ, :], vec[:, T0, :], ALU.subtract
                )
                nc.vector.tensor_scalar_add(vec[:, T1, :], vec[:, VAR, :], cfg.BN_EPS)
                nc.vector.reciprocal(vec[:, T0, :], vec[:, T1, :])
                nc.scalar.sqrt(vec[:, RSTD, :], vec[:, T0, :])
                nc.vector.tensor_tensor(
                    vec[:, SC, :], gb_sb[:, l, :], vec[:, RSTD, :], ALU.mult
                )
                nc.vector.tensor_tensor(
                    vec[:, T0, :], vec[:, MU, :], vec[:, SC, :], ALU.mult
                )
                nc.vector.tensor_tensor(
                    vec[:, SH, :], gb_sb[:, L + l, :], vec[:, T0, :], ALU.subtract
                )
                # broadcast scale|shift across partitions via ones-matmul
                bc_ps = psbc.tile([P, 2 * P], F32, tag="bc")
                nc.tensor.matmul(
                    out=bc_ps[:], lhsT=ones1[:], rhs=vec[:, SC : SH + 1, :],
                    start=True, stop=True,
                )
                screp = wp2.tile([P, 2, P], F32, tag="screp")
                nc.vector.tensor_copy(screp[:], bc_ps[:])

                # ---- BN apply + relu + residual (in place on agg_sb) ----
                nc.vector.tensor_tensor(
                    agg_sb[:],
                    agg_sb[:],
                    screp[:, 0:1, :].to_broadcast([P, TPC, D]),
                    ALU.mult,
                )
                nc.vector.tensor_tensor(
                    agg_sb[:],
                    agg_sb[:],
                    screp[:, 1:2, :].to_broadcast([P, TPC, D]),
                    ALU.add,
                )
                nc.scalar.activation(agg_sb[:], agg_sb[:], AF.Relu)
                nc.vector.tensor_tensor(x_sb[:], x_sb[:], agg_sb[:], ALU.add)

            nc.sync.dma_start(out_d[:].rearrange("(t p) f -> p t f", p=P), x_sb[:])

    nc.compile()
    return nc


_CACHE = {}


def _get_nc(cfg, T1s, T2s):
    key = (cfg.N, cfg.E, cfg.L, cfg.C, cfg.BPC, cfg.KG, cfg.L1, T1s, T2s)
    if key not in _CACHE:
        _CACHE[key] = _build(cfg, T1s, T2s)
    return _CACHE[key]


def run(cfg, inputs, trace=False):
    in_maps, meta = _preprocess(cfg, **inputs)
    nc = _get_nc(cfg, meta["T1s"], meta["T2s"])
    res = run_bass_kernel_spmd(nc, in_maps, core_ids=list(range(cfg.C)), trace=trace)
    newlocal = meta["newlocal"]
    xfull = np.empty((cfg.N, cfg.D), np.float32)
    for c in range(cfg.C):
        ids = np.arange(c * cfg.NSH, (c + 1) * cfg.NSH)
        xfull[ids] = res.results[c]["out"][newlocal[ids]]
    return xfull, res


def kernel(x, edge_index, W, b, gamma, beta):
    cfg = Cfg(N=50000, E=800000, D=128, L=3, C=8, bpc=7, kg=8)
    out, _ = run(
        cfg, dict(x=x, edge_index=edge_index, W=W, b=b, gamma=gamma, beta=beta)
    )
    return out
